# revision 40
# baseline (speedup 1.0000x reference)
"""Bass/Tile multi-head attention kernel builder for TRN2 (v3).

Per-core problem (core c handles batch b=c//2, head-group g=c%2):
  inputs:  xq, xk, xv [S, DIN] bf16     (batch b slices of q/k/v, host-cast)
           wq, wk, wv [DIN, DC] bf16    (column slice for this head group)
           wo [DC, DOUT] bf16           (row slice)
           bq, bk, bv [DC] f32
  output:  out [S, DOUT] f32  partial:  host sums the two head-group partials
           per batch and adds bo.

Math (per head h of H local heads, depth=64):
  xt   = X^T via DMA-xbar transpose loads          [DIN(p-major blocks), S]
  QT   = (wq_blk.T @ xt) + bq                       [DC, S]  f32r
  KT   = (wk_blk.T @ xt) + bk                       [DC, S]  f32r
  V    = (xt_chunk.T @ wv) + bv (+ ones col)        [S, DC(+1/head)] bf16
  ST   = KT_h.T @ QT_h   (64-partition contraction) [keys, q] per head
  E    = exp(ST * 1/sqrt(depth))  -> bf16           (logits O(10), no max-sub)
  OT   = E_chunk.T @ V_aug_h  (transposed-AV)       [q, depth+1] accum over keys
  O    = OT[:, :depth] / OT[:, depth]  -> bf16      (free-dim normalize)
  OTn  = O^T per head pair (PE transpose)           [DC, S] bf16
  out  = OTn.T @ wo                                 [S, DOUT] f32

The emission order is a hand-rolled software pipeline: the scalar engine
(exp over all S^2 logits) is the throughput floor, so score/exp work is
interleaved into the K/V projection phases and the per-sqt out/Q
projections are spread as PE filler inside the attention kt loop, keeping
both PE and ACT continuously fed.
"""

import math
from collections import deque
from contextlib import ExitStack

import concourse.mybir as mybir
from concourse import bacc
from concourse.masks import make_identity
from concourse.tile import TileContext

F32 = mybir.dt.float32
F32R = mybir.dt.float32r
BF16 = mybir.dt.bfloat16
P = 128
EXP = mybir.ActivationFunctionType.Exp


def build_mha_core(S=2048, DIN=1024, DC=512, DOUT=1024, H=8, depth=64,
                   SQT=512, num_devices=1, ablate="", q_bufs=2, ex_bufs=30,
                   st_bufs=4, xt_bufs=2):
    ablate = set(ablate.split(",")) if ablate else set()
    assert DC == H * depth and DC % P == 0 and DIN % P == 0 and S % SQT == 0
    NKT = S // P          # key chunks of 128
    NDIN = DIN // P       # input-dim k-tiles
    NDO = DC // P         # d_core blocks
    NSQT = S // SQT       # attention q tiles
    NSQC = SQT // P       # 128-query chunks per sqt
    scale = 1.0 / float(depth) ** 0.5

    nc = bacc.Bacc("TRN2", target_bir_lowering=False, debug=False,
                   num_devices=num_devices)
    xq = nc.dram_tensor("xq", [S, DIN], BF16, kind="ExternalInput")
    xk = nc.dram_tensor("xk", [S, DIN], BF16, kind="ExternalInput")
    xv = nc.dram_tensor("xv", [S, DIN], BF16, kind="ExternalInput")
    wq = nc.dram_tensor("wq", [DIN, DC], BF16, kind="ExternalInput")
    wk = nc.dram_tensor("wk", [DIN, DC], BF16, kind="ExternalInput")
    wv = nc.dram_tensor("wv", [DIN, DC], BF16, kind="ExternalInput")
    wo = nc.dram_tensor("wo", [DC, DOUT], BF16, kind="ExternalInput")
    bq = nc.dram_tensor("bq", [DC], F32, kind="ExternalInput")
    bk = nc.dram_tensor("bk", [DC], F32, kind="ExternalInput")
    bv = nc.dram_tensor("bv", [DC], F32, kind="ExternalInput")
    out = nc.dram_tensor("out", [S, DOUT], F32, kind="ExternalOutput")

    with TileContext(nc) as tc, ExitStack() as ctx:
        const = ctx.enter_context(tc.tile_pool(name="const", bufs=1))
        wpool = ctx.enter_context(tc.tile_pool(name="wpool", bufs=1))
        kvpool = ctx.enter_context(tc.tile_pool(name="kv", bufs=1))
        xtkv = ctx.enter_context(tc.tile_pool(name="xtkv", bufs=xt_bufs))
        xtq = ctx.enter_context(tc.tile_pool(name="xtq", bufs=xt_bufs))
        qpool = ctx.enter_context(tc.tile_pool(name="qp", bufs=q_bufs))
        expool = ctx.enter_context(tc.tile_pool(name="ex", bufs=ex_bufs))
        opool = ctx.enter_context(tc.tile_pool(name="op", bufs=2))
        misc = ctx.enter_context(tc.tile_pool(name="misc", bufs=2))
        ps_st = ctx.enter_context(tc.tile_pool(name="ps_st", bufs=st_bufs,
                                               space="PSUM"))
        ps_ot = ctx.enter_context(tc.tile_pool(name="ps_ot", bufs=1,
                                               space="PSUM"))
        ps_gen = ctx.enter_context(tc.tile_pool(name="ps_gen", bufs=2,
                                                space="PSUM"))

        ident = const.tile([P, P], BF16)
        make_identity(nc, ident)
        # warm the Exp activation table while the first DMAs are in flight
        warm = const.tile([1, 2], F32)
        nc.vector.memset(warm[:], 0.0)
        nc.scalar.activation(warm[0:1, 0:1], warm[0:1, 1:2], EXP)

        # ---- weights: direct bf16 DMA loads, no staging ----
        def load_weight(dram, kdim, ndim, name, split=False):
            w = wpool.tile([P, kdim // P, ndim], BF16, name=name)
            if split:
                # first output block loads first: unblocks the do=0 chain
                nc.sync.dma_start(
                    w[:, :, 0:P],
                    dram[:, 0:P].rearrange("(o p) n -> p o n", p=P))
                nc.sync.dma_start(
                    w[:, :, P:ndim],
                    dram[:, P:].rearrange("(o p) n -> p o n", p=P))
            else:
                nc.sync.dma_start(
                    w[:], dram[:, :].rearrange("(o p) n -> p o n", p=P))
            return w

        KT = kvpool.tile([P, NDO, S], F32R)
        V = kvpool.tile([P, NKT, H, depth + 1], BF16)
        nc.vector.memset(V[:, :, :, depth:depth + 1], 1.0)

        # ---------------- emitters ----------------
        def kproj_half(xt, st_i, do, half):
            ps = ps_gen.tile([P, 256], F32, tag="gen", name="pskh")
            for kt in range(NDIN):
                nc.tensor.matmul(
                    ps[:], wk_sb[:, kt, do * P:(do + 1) * P],
                    xt[:, kt, half * 256:(half + 1) * 256],
                    start=(kt == 0), stop=(kt == NDIN - 1))
            nc.vector.tensor_scalar_add(
                KT[:, do, st_i * 512 + half * 256:st_i * 512 + half * 256
                   + 256], ps[:], bk_sb[:, do:do + 1])

        def kproj_do0(st_i, split=False):
            xt = xtkv.tile([P, NDIN, 512], BF16, tag="xt", name="xtk")
            rows = xk[st_i * 512:(st_i + 1) * 512, :]
            if split:
                # row-split so the first scores unblock after half a chunk
                nc.sync.dma_start_transpose(xt[:, :, 0:256], rows[0:256, :])
                kproj_half(xt, st_i, 0, 0)
                scores_exp(0, 0, 4 * st_i)
                scores_exp(0, 0, 4 * st_i + 1)
                nc.sync.dma_start_transpose(xt[:, :, 256:512],
                                            rows[256:512, :])
                kproj_half(xt, st_i, 0, 1)
            else:
                nc.sync.dma_start_transpose(xt[:], rows)
                kproj_rest(xt, st_i, dos=(0,))
            return xt

        def kproj_rest(xt, st_i, dos=(1, 2, 3)):
            for do in dos:
                ps = ps_gen.tile([P, 512], F32, tag="gen", name="psk")
                for kt in range(NDIN):
                    nc.tensor.matmul(
                        ps[:], wk_sb[:, kt, do * P:(do + 1) * P], xt[:, kt, :],
                        start=(kt == 0), stop=(kt == NDIN - 1))
                nc.vector.tensor_scalar_add(
                    KT[:, do, st_i * 512:(st_i + 1) * 512], ps[:],
                    bk_sb[:, do:do + 1])

        def vproj_chunk(st_i, after_sc=None):
            xt = xtkv.tile([P, NDIN, 512], BF16, tag="xt", name="xtv")
            nc.sync.dma_start_transpose(
                xt[:], xv[st_i * 512:(st_i + 1) * 512, :])
            for sc in range(4):
                ps = ps_gen.tile([P, 512], F32, tag="gen", name="psv")
                for kt in range(NDIN):
                    nc.tensor.matmul(
                        ps[:], xt[:, kt, sc * P:(sc + 1) * P], wv_sb[:, kt, :],
                        start=(kt == 0), stop=(kt == NDIN - 1))
                chunk = st_i * 4 + sc
                nc.vector.tensor_tensor(
                    V[:, chunk, :, 0:depth],
                    ps[:].rearrange("p (h d) -> p h d", h=H),
                    bv_bc[:].rearrange("p (h d) -> p h d", h=H),
                    mybir.AluOpType.add)
                if after_sc is not None:
                    after_sc(chunk)

        QTs = {}

        def qproj_load(sqt, split_first=False):
            xt = xtq.tile([P, NDIN, SQT], BF16, tag="xt", name="xtq")
            rows = xq[sqt * SQT:(sqt + 1) * SQT, :]
            if split_first:
                # load the first k-tile separately so chain kt=0 can start
                # before the bulk of the transpose-load finishes
                nc.sync.dma_start_transpose(xt[:, 0:1, :], rows[:, 0:P])
                nc.sync.dma_start_transpose(xt[:, 1:NDIN, :], rows[:, P:])
            else:
                nc.sync.dma_start_transpose(xt[:], rows)
            QTs[sqt] = (qpool.tile([P, NDO, SQT], F32R, tag="qt", name="qt"),
                        xt)

        def qproj_chain(sqt, do):
            QT, xt = QTs[sqt]
            ps = ps_gen.tile([P, 512], F32, tag="gen", name="psq")
            for kt in range(NDIN):
                nc.tensor.matmul(
                    ps[:], wq_sb[:, kt, do * P:(do + 1) * P], xt[:, kt, :],
                    start=(kt == 0), stop=(kt == NDIN - 1))
            nc.vector.tensor_scalar_add(QT[:, do, :], ps[:],
                                        bq_sb[:, do:do + 1])

        def qproj_half(sqt, do, half):
            # finer-grained filler: half the free dim per chain
            QT, xt = QTs[sqt]
            ps = ps_gen.tile([P, 256], F32, tag="gen", name="psqh")
            for kt in range(NDIN):
                nc.tensor.matmul(
                    ps[:], wq_sb[:, kt, do * P:(do + 1) * P],
                    xt[:, kt, half * 256:(half + 1) * 256],
                    start=(kt == 0), stop=(kt == NDIN - 1))
            nc.vector.tensor_scalar_add(
                QT[:, do, half * 256:(half + 1) * 256], ps[:],
                bq_sb[:, do:do + 1])

        ex_map = {}
        ot_map = {}
        OTns = {}

        # Schraudolph bit-trick exp for the DVE: exp(s*x) ~=
        # bitcast_bf16(int16(A*x + B)) with A = 128*s/ln2, B = 127*128 - c.
        # ~+-3% per weight, self-consistent through the softmax denominator
        # (it sums the same approximated values). Used on a fraction of key
        # tiles to offload the scalar engine, which is the throughput floor.
        EXPA = 128.0 * scale / math.log(2.0)
        EXPB = 127.0 * 128.0 - 7.42

        def scores_exp(sqt, hp, kt, dve=False):
            # per-head one-bank st tiles: a 4-slot rotation in the same 4
            # PSUM banks doubles the scores->exp pipeline elasticity, and
            # per-head exp ops allow a finer ACT/DVE split.
            QT = QTs[sqt][0]
            ex = expool.tile([P, 2, 512], BF16, tag="ex", name="ex")
            for hi, h in enumerate((2 * hp, 2 * hp + 1)):
                st = ps_st.tile([P, 512], F32, name="st")
                p0 = (h % 2) * 64
                nc.tensor.matmul(
                    st[:],
                    KT[p0:p0 + 64, hp, kt * P:(kt + 1) * P],
                    QT[p0:p0 + 64, hp, :],
                    start=True, stop=True)
                on_dve = dve and (hi == 1 or kt % 8 == 7)
                if on_dve:
                    nc.vector.tensor_scalar(
                        ex[:, hi, :].bitcast(mybir.dt.int16), st[:],
                        EXPA, EXPB,
                        mybir.AluOpType.mult, mybir.AluOpType.add)
                else:
                    nc.scalar.activation(ex[:, hi, :], st[:], EXP,
                                         scale=scale)
            ex_map[(sqt, hp, kt)] = ex

        def av_t(sqt, hp, kt):
            if kt == 0:
                ot_map[(sqt, hp)] = [
                    ps_ot.tile([P, NSQC, P], F32, name=f"ot{i}")
                    for i in range(2)]
            ot_ps = ot_map[(sqt, hp)]
            ex = ex_map.pop((sqt, hp, kt))
            for hi, h in enumerate((2 * hp, 2 * hp + 1)):
                for qc in range(NSQC):
                    # start zeroes the whole 2KB PSUM zero-region (bank), so
                    # only the very first matmul into each head's bank starts
                    # the group; all four qc regions then accumulate onto
                    # zeros.
                    nc.tensor.matmul(
                        ot_ps[hi][:, qc, 0:depth + 1],
                        ex[:, hi, qc * P:(qc + 1) * P],
                        V[:, kt, h, :],
                        start=(kt == 0 and qc == 0),
                        stop=(kt == NKT - 1 and qc == NSQC - 1),
                        skip_group_check=True)

        def norm_transp(sqt, hp, oproj_after_qc=False):
            if hp == 0:
                OTns[sqt] = opool.tile([P, NDO, SQT], BF16, tag="otn",
                                       name="otn")
            OTn = OTns[sqt]
            ot_ps = ot_map.pop((sqt, hp))
            O_sb = opool.tile([P, NSQC, 2, depth], BF16, tag="osb",
                              name="osb")
            for hi in range(2):
                rec = misc.tile([P, NSQC, 1], F32, tag="rec", name="rec")
                nc.vector.reciprocal(rec[:],
                                     ot_ps[hi][:, :, depth:depth + 1])
                nc.vector.tensor_tensor(
                    O_sb[:, :, hi, :], ot_ps[hi][:, :, 0:depth],
                    rec[:].to_broadcast((P, NSQC, depth)),
                    mybir.AluOpType.mult)
            for qc in range(NSQC):
                tp = ps_gen.tile([P, P], BF16, tag="gen", name="tp")
                nc.tensor.transpose(tp[:], O_sb[:, qc, :, :], ident[:])
                nc.vector.tensor_copy(
                    OTn[:, hp, qc * P:(qc + 1) * P], tp[:])
                if oproj_after_qc:
                    for do in range(DOUT // 512):
                        oproj_chain(sqt, do, qc, direct=True)

        def oproj_chain(sqt, do, sc, direct=False):
            OTn = OTns[sqt]
            ps = ps_gen.tile([P, 512], F32, tag="gen", name="pso")
            for hh in range(NDO):
                nc.tensor.matmul(
                    ps[:], OTn[:, hh, sc * P:(sc + 1) * P],
                    wo_sb[:, hh, do * 512:(do + 1) * 512],
                    start=(hh == 0), stop=(hh == NDO - 1))
            r0 = sqt * SQT + sc * P
            osb = misc.tile([P, 512], F32, tag="osb2", name="osb2")
            nc.vector.tensor_copy(osb[:], ps[:])
            nc.sync.dma_start(out[r0:r0 + P, do * 512:(do + 1) * 512],
                              osb[:])

        # ---------------- schedule ----------------
        # Loads ordered so the first scores+exp are unblocked ASAP: the
        # critical DMA chain is xtq-T, wq, wk, xtk-half0-T.
        wq_sb = load_weight(wq, DIN, DC, "wq_sb")
        qproj_load(0, split_first=True)
        bq_sb = const.tile([P, NDO], F32)
        nc.sync.dma_start(bq_sb[:], bq[:].rearrange("(o p) -> p o", p=P))
        qproj_chain(0, 0)
        wk_sb = load_weight(wk, DIN, DC, "wk_sb")
        bk_sb = const.tile([P, NDO], F32)
        nc.sync.dma_start(bk_sb[:], bk[:].rearrange("(o p) -> p o", p=P))

        # Phase B: K-proj, with sqt0 hp0 (and first hp1) scores+exp fused in.
        for st_i in range(4):
            xt = kproj_do0(st_i, split=(st_i == 0))
            if st_i == 0:
                scores_exp(0, 0, 2)
                scores_exp(0, 0, 3)
                kproj_rest(xt, 0, dos=(1,))
                for do in range(1, NDO):
                    qproj_chain(0, do)
                kproj_rest(xt, 0, dos=(2, 3))
            else:
                for kt in range(4 * st_i, 4 * st_i + 4):
                    scores_exp(0, 0, kt)
                kproj_rest(xt, st_i)
            if st_i == 3:
                for kt in range(0, 8):
                    scores_exp(0, 1, kt)
        # Phase C: V-proj groups; consume hp0 via AV-T as each V chunk
        # lands, keep the exp stream fed with hp1/hp2 scores.
        wv_sb = load_weight(wv, DIN, DC, "wv_sb")
        bv_st = const.tile([1, DC], F32)
        nc.sync.dma_start(bv_st[0:1, :], bv[:][None, :])
        bv_bc = const.tile([P, DC], F32)
        nc.gpsimd.partition_broadcast(bv_bc[:], bv_st[0:1, :])
        wo_sb = load_weight(wo, DC, DOUT, "wo_sb")
        C_SCORES = [(1, kt) for kt in range(8, 16)] + \
                   [(2, kt) for kt in range(0, 14)]
        for st_i in range(4):
            def consume(chunk):
                av_t(0, 0, chunk)
                npop = 2 if chunk < 8 else 1
                for _ in range(npop):
                    if C_SCORES:
                        hp_n, kt = C_SCORES.pop(0)
                        scores_exp(0, hp_n, kt, dve=(kt % 2 == 1))
            vproj_chunk(st_i, after_sc=consume)
        # Phase D: finish sqt0 (hp1..hp3), qproj(1) as filler.
        qproj_load(1)
        filler = deque()
        for do in range(NDO):
            filler.append((qproj_chain, (1, do)))
        norm_transp(0, 0)
        D_SCORES = [(2, 14), (2, 15)] + [(3, kt) for kt in range(NKT)]
        for kt in range(NKT):
            av_t(0, 1, kt)
            for _ in range(2 if kt < 2 else 1):
                if D_SCORES:
                    hp_n, kt_n = D_SCORES.pop(0)
                    scores_exp(0, hp_n, kt_n, dve=(kt_n % 2 == 1))
            if kt % 4 == 1 and filler:
                f, a = filler.popleft()
                f(*a)
        norm_transp(0, 1)
        for kt in range(NKT):
            av_t(0, 2, kt)
            if kt % 4 == 1 and filler:
                f, a = filler.popleft()
                f(*a)
        norm_transp(0, 2)
        for kt in range(NKT - 1):
            av_t(0, 3, kt)
            if kt % 4 == 1 and filler:
                f, a = filler.popleft()
                f(*a)
        while filler:
            f, a = filler.popleft()
            f(*a)

        # Phase E: steady flattened stream over (hp, kt); each hp's last
        # AV-T and normalize fire after the NEXT hp's first scores so the
        # exp stream never waits on the norm/transpose block. Previous
        # sqt's out-proj and next sqt's Q-proj interleave as PE filler.
        # AV-T at lag-2: scores(kt) waiting on the st-buffer rotation
        # already implies exp(kt-2) completed, so an AV-T emitted two score
        # steps behind never head-of-line blocks the PE queue.
        pend = deque([(0, 3, NKT - 1)])  # phase-D leftover AV-T
        for sqt in range(1, NSQT):
            last = sqt == NSQT - 1
            if not last:
                qproj_load(sqt + 1)
            filler = deque()
            if not last:
                for do in range(NDO):
                    for half in range(2):
                        filler.append((qproj_half, (sqt + 1, do, half)))
            for do in range(DOUT // 512):
                for sc in range(NSQC):
                    filler.append((oproj_chain, (sqt - 1, do, sc)))
            for hp in range(H // 2):
                for kt in range(NKT):
                    scores_exp(sqt, hp, kt, dve=(kt % 2 == 1))
                    pend.append((sqt, hp, kt))
                    if len(pend) > 2:
                        done = pend.popleft()
                        av_t(*done)
                        if done[2] == NKT - 1:
                            norm_transp(done[0], done[1])
                    if kt % 4 == 1 and filler:
                        f, a = filler.popleft()
                        f(*a)
            while filler:
                f, a = filler.popleft()
                f(*a)
        while pend:
            done = pend.popleft()
            av_t(*done)
            if done[2] == NKT - 1 and done != (NSQT - 1, H // 2 - 1, NKT - 1):
                norm_transp(done[0], done[1])
        norm_transp(NSQT - 1, H // 2 - 1, oproj_after_qc=True)

    nc.compile()
    return nc


# ---------------------------------------------------------------------------
# Host-side wrapper: shard across 8 NeuronCores, run SPMD, gather.
# Core c handles batch b = c // 2 and head-group g = c % 2 (8 of 16 heads,
# i.e. columns [g*512, (g+1)*512) of Wq/Wk/Wv and rows of Wo).
# ---------------------------------------------------------------------------

import numpy as np
import ml_dtypes

from concourse.bass_utils import run_bass_kernel_spmd

_NC = None
_BF16 = ml_dtypes.bfloat16


def _get_nc():
    global _NC
    if _NC is None:
        _NC = build_mha_core(S=2048, DIN=1024, DC=512, DOUT=1024, H=8,
                             depth=64, num_devices=8)
    return _NC


def _in_maps(q, k, v, Wq, bq, Wk, bk, Wv, bv, Wo, bo):
    f32 = np.float32
    maps = []
    qb = [np.ascontiguousarray(np.asarray(q[b], dtype=f32).astype(_BF16))
          for b in range(4)]
    kb = [np.ascontiguousarray(np.asarray(k[b], dtype=f32).astype(_BF16))
          for b in range(4)]
    vb = [np.ascontiguousarray(np.asarray(v[b], dtype=f32).astype(_BF16))
          for b in range(4)]
    Wq = np.asarray(Wq, dtype=f32)
    Wk = np.asarray(Wk, dtype=f32)
    Wv = np.asarray(Wv, dtype=f32)
    Wo = np.asarray(Wo, dtype=f32)
    for c in range(8):
        b, g = c // 2, c % 2
        sl = slice(g * 512, (g + 1) * 512)
        maps.append({
            "xq": qb[b],
            "xk": kb[b],
            "xv": vb[b],
            "wq": np.ascontiguousarray(Wq[:, sl].astype(_BF16)),
            "wk": np.ascontiguousarray(Wk[:, sl].astype(_BF16)),
            "wv": np.ascontiguousarray(Wv[:, sl].astype(_BF16)),
            "wo": np.ascontiguousarray(Wo[sl, :].astype(_BF16)),
            "bq": np.ascontiguousarray(bq[sl], dtype=f32),
            "bk": np.ascontiguousarray(bk[sl], dtype=f32),
            "bv": np.ascontiguousarray(bv[sl], dtype=f32),
        })
    return maps


def _gather(results, bo):
    out = np.empty((4, 2048, 1024), dtype=np.float32)
    bo32 = np.asarray(bo, dtype=np.float32)
    for b in range(4):
        out[b] = results[2 * b]["out"] + results[2 * b + 1]["out"] + bo32
    return out


def kernel(q, k, v, Wq, bq, Wk, bk, Wv, bv, Wo, bo, _trace=False):
    nc = _get_nc()
    res = run_bass_kernel_spmd(
        nc, _in_maps(q, k, v, Wq, bq, Wk, bk, Wv, bv, Wo, bo),
        core_ids=list(range(8)), trace=_trace)
    out = _gather(res.results, bo)
    if _trace:
        kernel.last_results = res
    return out


# revision 41
# speedup vs baseline: 1.0182x; 1.0182x over previous
"""Bass/Tile multi-head attention kernel builder for TRN2 (v3).

Per-core problem (core c handles batch b=c//2, head-group g=c%2):
  inputs:  xq, xk, xv [S, DIN] bf16     (batch b slices of q/k/v, host-cast)
           wq, wk, wv [DIN, DC] bf16    (column slice for this head group)
           wo [DC, DOUT] bf16           (row slice)
           bq, bk, bv [DC] f32
  output:  out [S, DOUT] f32  partial:  host sums the two head-group partials
           per batch and adds bo.

Math (per head h of H local heads, depth=64):
  xt   = X^T via DMA-xbar transpose loads          [DIN(p-major blocks), S]
  QT   = (wq_blk.T @ xt) + bq                       [DC, S]  f32r
  KT   = (wk_blk.T @ xt) + bk                       [DC, S]  f32r
  V    = (xt_chunk.T @ wv) + bv (+ ones col)        [S, DC(+1/head)] bf16
  ST   = KT_h.T @ QT_h   (64-partition contraction) [keys, q] per head
  E    = exp(ST * 1/sqrt(depth))  -> bf16           (logits O(10), no max-sub)
  OT   = E_chunk.T @ V_aug_h  (transposed-AV)       [q, depth+1] accum over keys
  O    = OT[:, :depth] / OT[:, depth]  -> bf16      (free-dim normalize)
  OTn  = O^T per head pair (PE transpose)           [DC, S] bf16
  out  = OTn.T @ wo                                 [S, DOUT] f32

The emission order is a hand-rolled software pipeline: the scalar engine
(exp over all S^2 logits) is the throughput floor, so score/exp work is
interleaved into the K/V projection phases and the per-sqt out/Q
projections are spread as PE filler inside the attention kt loop, keeping
both PE and ACT continuously fed.
"""

import math
from collections import deque
from contextlib import ExitStack

import concourse.mybir as mybir
from concourse import bacc
from concourse.masks import make_identity
from concourse.tile import TileContext

F32 = mybir.dt.float32
F32R = mybir.dt.float32r
BF16 = mybir.dt.bfloat16
P = 128
EXP = mybir.ActivationFunctionType.Exp


def build_mha_core(S=2048, DIN=1024, DC=512, DOUT=1024, H=8, depth=64,
                   SQT=512, num_devices=1, ablate="", q_bufs=2, ex_bufs=30,
                   st_bufs=4, xt_bufs=2):
    ablate = set(ablate.split(",")) if ablate else set()
    assert DC == H * depth and DC % P == 0 and DIN % P == 0 and S % SQT == 0
    NKT = S // P          # key chunks of 128
    NDIN = DIN // P       # input-dim k-tiles
    NDO = DC // P         # d_core blocks
    NSQT = S // SQT       # attention q tiles
    NSQC = SQT // P       # 128-query chunks per sqt
    scale = 1.0 / float(depth) ** 0.5

    nc = bacc.Bacc("TRN2", target_bir_lowering=False, debug=False,
                   num_devices=num_devices)
    xq = nc.dram_tensor("xq", [S, DIN], BF16, kind="ExternalInput")
    xk = nc.dram_tensor("xk", [S, DIN], BF16, kind="ExternalInput")
    xv = nc.dram_tensor("xv", [S, DIN], BF16, kind="ExternalInput")
    wq = nc.dram_tensor("wq", [DIN, DC], BF16, kind="ExternalInput")
    wk = nc.dram_tensor("wk", [DIN, DC], BF16, kind="ExternalInput")
    wv = nc.dram_tensor("wv", [DIN, DC], BF16, kind="ExternalInput")
    wo = nc.dram_tensor("wo", [DC, DOUT], BF16, kind="ExternalInput")
    bq = nc.dram_tensor("bq", [DC], F32, kind="ExternalInput")
    bk = nc.dram_tensor("bk", [DC], F32, kind="ExternalInput")
    bv = nc.dram_tensor("bv", [DC], F32, kind="ExternalInput")
    out = nc.dram_tensor("out", [S, DOUT], F32, kind="ExternalOutput")

    with TileContext(nc) as tc, ExitStack() as ctx:
        const = ctx.enter_context(tc.tile_pool(name="const", bufs=1))
        wpool = ctx.enter_context(tc.tile_pool(name="wpool", bufs=1))
        kvpool = ctx.enter_context(tc.tile_pool(name="kv", bufs=1))
        xtkv = ctx.enter_context(tc.tile_pool(name="xtkv", bufs=xt_bufs))
        xtq = ctx.enter_context(tc.tile_pool(name="xtq", bufs=xt_bufs))
        qpool = ctx.enter_context(tc.tile_pool(name="qp", bufs=q_bufs))
        expool = ctx.enter_context(tc.tile_pool(name="ex", bufs=ex_bufs))
        opool = ctx.enter_context(tc.tile_pool(name="op", bufs=2))
        misc = ctx.enter_context(tc.tile_pool(name="misc", bufs=2))
        ps_st = ctx.enter_context(tc.tile_pool(name="ps_st", bufs=st_bufs,
                                               space="PSUM"))
        ps_ot = ctx.enter_context(tc.tile_pool(name="ps_ot", bufs=1,
                                               space="PSUM"))
        ps_gen = ctx.enter_context(tc.tile_pool(name="ps_gen", bufs=2,
                                                space="PSUM"))

        ident = const.tile([P, P], BF16)
        make_identity(nc, ident)
        # warm the Exp activation table while the first DMAs are in flight
        warm = const.tile([1, 2], F32)
        nc.vector.memset(warm[:], 0.0)
        nc.scalar.activation(warm[0:1, 0:1], warm[0:1, 1:2], EXP)

        # ---- weights: direct bf16 DMA loads, no staging ----
        def load_weight(dram, kdim, ndim, name, split=False):
            w = wpool.tile([P, kdim // P, ndim], BF16, name=name)
            if split:
                # first output block loads first: unblocks the do=0 chain
                nc.sync.dma_start(
                    w[:, :, 0:P],
                    dram[:, 0:P].rearrange("(o p) n -> p o n", p=P))
                nc.sync.dma_start(
                    w[:, :, P:ndim],
                    dram[:, P:].rearrange("(o p) n -> p o n", p=P))
            else:
                nc.sync.dma_start(
                    w[:], dram[:, :].rearrange("(o p) n -> p o n", p=P))
            return w

        KT = kvpool.tile([P, NDO, S], F32R)
        V = kvpool.tile([P, NKT, H, depth + 1], BF16)
        nc.vector.memset(V[:, :, :, depth:depth + 1], 1.0)

        # ---------------- emitters ----------------
        def kproj_half(xt, st_i, do, half):
            ps = ps_gen.tile([P, 256], F32, tag="gen", name="pskh")
            for kt in range(NDIN):
                nc.tensor.matmul(
                    ps[:], wk_sb[:, kt, do * P:(do + 1) * P],
                    xt[:, kt, half * 256:(half + 1) * 256],
                    start=(kt == 0), stop=(kt == NDIN - 1))
            nc.vector.tensor_scalar_add(
                KT[:, do, st_i * 512 + half * 256:st_i * 512 + half * 256
                   + 256], ps[:], bk_sb[:, do:do + 1])

        def kproj_do0(st_i, split=False):
            xt = xtkv.tile([P, NDIN, 512], BF16, tag="xt", name="xtk")
            rows = xk[st_i * 512:(st_i + 1) * 512, :]
            if split:
                # row-split so the first scores unblock after half a chunk
                nc.sync.dma_start_transpose(xt[:, :, 0:256], rows[0:256, :])
                kproj_half(xt, st_i, 0, 0)
                scores_exp(0, 0, 4 * st_i)
                scores_exp(0, 0, 4 * st_i + 1)
                nc.sync.dma_start_transpose(xt[:, :, 256:512],
                                            rows[256:512, :])
                kproj_half(xt, st_i, 0, 1)
            else:
                nc.sync.dma_start_transpose(xt[:], rows)
                kproj_rest(xt, st_i, dos=(0,))
            return xt

        def kproj_rest(xt, st_i, dos=(1, 2, 3)):
            for do in dos:
                ps = ps_gen.tile([P, 512], F32, tag="gen", name="psk")
                for kt in range(NDIN):
                    nc.tensor.matmul(
                        ps[:], wk_sb[:, kt, do * P:(do + 1) * P], xt[:, kt, :],
                        start=(kt == 0), stop=(kt == NDIN - 1))
                nc.vector.tensor_scalar_add(
                    KT[:, do, st_i * 512:(st_i + 1) * 512], ps[:],
                    bk_sb[:, do:do + 1])

        def vproj_chunk(st_i, after_sc=None):
            xt = xtkv.tile([P, NDIN, 512], BF16, tag="xt", name="xtv")
            nc.sync.dma_start_transpose(
                xt[:], xv[st_i * 512:(st_i + 1) * 512, :])
            for sc in range(4):
                ps = ps_gen.tile([P, 512], F32, tag="gen", name="psv")
                for kt in range(NDIN):
                    nc.tensor.matmul(
                        ps[:], xt[:, kt, sc * P:(sc + 1) * P], wv_sb[:, kt, :],
                        start=(kt == 0), stop=(kt == NDIN - 1))
                chunk = st_i * 4 + sc
                nc.vector.tensor_tensor(
                    V[:, chunk, :, 0:depth],
                    ps[:].rearrange("p (h d) -> p h d", h=H),
                    bv_bc[:].rearrange("p (h d) -> p h d", h=H),
                    mybir.AluOpType.add)
                if after_sc is not None:
                    after_sc(chunk)

        QTs = {}

        def qproj_load(sqt, split_first=False):
            xt = xtq.tile([P, NDIN, SQT], BF16, tag="xt", name="xtq")
            rows = xq[sqt * SQT:(sqt + 1) * SQT, :]
            if split_first:
                # load the first k-tile separately so chain kt=0 can start
                # before the bulk of the transpose-load finishes
                nc.sync.dma_start_transpose(xt[:, 0:1, :], rows[:, 0:P])
                nc.sync.dma_start_transpose(xt[:, 1:NDIN, :], rows[:, P:])
            else:
                nc.sync.dma_start_transpose(xt[:], rows)
            QTs[sqt] = (qpool.tile([P, NDO, SQT], F32R, tag="qt", name="qt"),
                        xt)

        def qproj_chain(sqt, do):
            QT, xt = QTs[sqt]
            ps = ps_gen.tile([P, 512], F32, tag="gen", name="psq")
            for kt in range(NDIN):
                nc.tensor.matmul(
                    ps[:], wq_sb[:, kt, do * P:(do + 1) * P], xt[:, kt, :],
                    start=(kt == 0), stop=(kt == NDIN - 1))
            nc.vector.tensor_scalar_add(QT[:, do, :], ps[:],
                                        bq_sb[:, do:do + 1])

        def qproj_half(sqt, do, half):
            # finer-grained filler: half the free dim per chain
            QT, xt = QTs[sqt]
            ps = ps_gen.tile([P, 256], F32, tag="gen", name="psqh")
            for kt in range(NDIN):
                nc.tensor.matmul(
                    ps[:], wq_sb[:, kt, do * P:(do + 1) * P],
                    xt[:, kt, half * 256:(half + 1) * 256],
                    start=(kt == 0), stop=(kt == NDIN - 1))
            nc.vector.tensor_scalar_add(
                QT[:, do, half * 256:(half + 1) * 256], ps[:],
                bq_sb[:, do:do + 1])

        ex_map = {}
        ot_map = {}
        OTns = {}

        # Schraudolph bit-trick exp for the DVE: exp(s*x) ~=
        # bitcast_bf16(int16(A*x + B)) with A = 128*s/ln2, B = 127*128 - c.
        # ~+-3% per weight, self-consistent through the softmax denominator
        # (it sums the same approximated values). Used on a fraction of key
        # tiles to offload the scalar engine, which is the throughput floor.
        EXPA = 128.0 * scale / math.log(2.0)
        EXPB = 127.0 * 128.0 - 7.42

        def scores_exp(sqt, hp, kt, dve=False):
            # per-head one-bank st tiles: a 4-slot rotation in the same 4
            # PSUM banks doubles the scores->exp pipeline elasticity, and
            # per-head exp ops allow a finer ACT/DVE split.
            QT = QTs[sqt][0]
            ex = expool.tile([P, 2, 512], BF16, tag="ex", name="ex")
            for hi, h in enumerate((2 * hp, 2 * hp + 1)):
                st = ps_st.tile([P, 512], F32, name="st")
                p0 = (h % 2) * 64
                nc.tensor.matmul(
                    st[:],
                    KT[p0:p0 + 64, hp, kt * P:(kt + 1) * P],
                    QT[p0:p0 + 64, hp, :],
                    start=True, stop=True)
                on_dve = dve and (hi == 1 or kt % 8 == 7)
                if on_dve:
                    nc.vector.tensor_scalar(
                        ex[:, hi, :].bitcast(mybir.dt.int16), st[:],
                        EXPA, EXPB,
                        mybir.AluOpType.mult, mybir.AluOpType.add)
                else:
                    nc.scalar.activation(ex[:, hi, :], st[:], EXP,
                                         scale=scale)
            ex_map[(sqt, hp, kt)] = ex

        def av_t(sqt, hp, kt):
            if kt == 0:
                ot_map[(sqt, hp)] = [
                    ps_ot.tile([P, NSQC, P], F32, name=f"ot{i}")
                    for i in range(2)]
            ot_ps = ot_map[(sqt, hp)]
            ex = ex_map.pop((sqt, hp, kt))
            for hi, h in enumerate((2 * hp, 2 * hp + 1)):
                for qc in range(NSQC):
                    # start zeroes the whole 2KB PSUM zero-region (bank), so
                    # only the very first matmul into each head's bank starts
                    # the group; all four qc regions then accumulate onto
                    # zeros.
                    nc.tensor.matmul(
                        ot_ps[hi][:, qc, 0:depth + 1],
                        ex[:, hi, qc * P:(qc + 1) * P],
                        V[:, kt, h, :],
                        start=(kt == 0 and qc == 0),
                        stop=(kt == NKT - 1 and qc == NSQC - 1),
                        skip_group_check=True)

        def norm_transp(sqt, hp, oproj_after_qc=False):
            if hp == 0:
                OTns[sqt] = opool.tile([P, NDO, SQT], BF16, tag="otn",
                                       name="otn")
            OTn = OTns[sqt]
            ot_ps = ot_map.pop((sqt, hp))
            O_sb = opool.tile([P, NSQC, 2, depth], BF16, tag="osb",
                              name="osb")
            for hi in range(2):
                rec = misc.tile([P, NSQC, 1], F32, tag="rec", name="rec")
                nc.vector.reciprocal(rec[:],
                                     ot_ps[hi][:, :, depth:depth + 1])
                nc.vector.tensor_tensor(
                    O_sb[:, :, hi, :], ot_ps[hi][:, :, 0:depth],
                    rec[:].to_broadcast((P, NSQC, depth)),
                    mybir.AluOpType.mult)
            for qc in range(NSQC):
                tp = ps_gen.tile([P, P], BF16, tag="gen", name="tp")
                nc.tensor.transpose(tp[:], O_sb[:, qc, :, :], ident[:])
                nc.vector.tensor_copy(
                    OTn[:, hp, qc * P:(qc + 1) * P], tp[:])
                if oproj_after_qc:
                    for do in range(DOUT // 512):
                        oproj_chain(sqt, do, qc, direct=True)

        def oproj_chain(sqt, do, sc, direct=False):
            OTn = OTns[sqt]
            ps = ps_gen.tile([P, 512], F32, tag="gen", name="pso")
            for hh in range(NDO):
                nc.tensor.matmul(
                    ps[:], OTn[:, hh, sc * P:(sc + 1) * P],
                    wo_sb[:, hh, do * 512:(do + 1) * 512],
                    start=(hh == 0), stop=(hh == NDO - 1))
            r0 = sqt * SQT + sc * P
            osb = misc.tile([P, 512], F32, tag="osb2", name="osb2")
            nc.vector.tensor_copy(osb[:], ps[:])
            nc.sync.dma_start(out[r0:r0 + P, do * 512:(do + 1) * 512],
                              osb[:])

        # ---------------- schedule ----------------
        # Loads ordered so the first scores+exp are unblocked ASAP: the
        # critical DMA chain is xtq-T, wq, wk, xtk-half0-T.
        qproj_load(0, split_first=True)
        wq_sb = load_weight(wq, DIN, DC, "wq_sb")
        bq_sb = const.tile([P, NDO], F32)
        nc.sync.dma_start(bq_sb[:], bq[:].rearrange("(o p) -> p o", p=P))
        qproj_chain(0, 0)
        wk_sb = load_weight(wk, DIN, DC, "wk_sb")
        bk_sb = const.tile([P, NDO], F32)
        nc.sync.dma_start(bk_sb[:], bk[:].rearrange("(o p) -> p o", p=P))

        # Phase B: K-proj, with sqt0 hp0 (and first hp1) scores+exp fused in.
        for st_i in range(4):
            xt = kproj_do0(st_i, split=(st_i == 0))
            if st_i == 0:
                scores_exp(0, 0, 2)
                scores_exp(0, 0, 3)
                kproj_rest(xt, 0, dos=(1,))
                for do in range(1, NDO):
                    qproj_chain(0, do)
                kproj_rest(xt, 0, dos=(2, 3))
            else:
                for kt in range(4 * st_i, 4 * st_i + 4):
                    scores_exp(0, 0, kt)
                kproj_rest(xt, st_i)
            if st_i == 3:
                for kt in range(0, 8):
                    scores_exp(0, 1, kt)
        # Phase C: V-proj groups; consume hp0 via AV-T as each V chunk
        # lands, keep the exp stream fed with hp1/hp2 scores.
        wv_sb = load_weight(wv, DIN, DC, "wv_sb")
        bv_st = const.tile([1, DC], F32)
        nc.sync.dma_start(bv_st[0:1, :], bv[:][None, :])
        bv_bc = const.tile([P, DC], F32)
        nc.gpsimd.partition_broadcast(bv_bc[:], bv_st[0:1, :])
        wo_sb = load_weight(wo, DC, DOUT, "wo_sb")
        C_SCORES = [(1, kt) for kt in range(8, 16)] + \
                   [(2, kt) for kt in range(0, 14)]
        for st_i in range(4):
            def consume(chunk):
                av_t(0, 0, chunk)
                npop = 2 if chunk < 8 else 1
                for _ in range(npop):
                    if C_SCORES:
                        hp_n, kt = C_SCORES.pop(0)
                        scores_exp(0, hp_n, kt)
            vproj_chunk(st_i, after_sc=consume)
        # Phase D: finish sqt0 (hp1..hp3), qproj(1) as filler.
        qproj_load(1)
        filler = deque()
        for do in range(NDO):
            filler.append((qproj_chain, (1, do)))
        norm_transp(0, 0)
        D_SCORES = [(2, 14), (2, 15)] + [(3, kt) for kt in range(NKT)]
        for kt in range(NKT):
            av_t(0, 1, kt)
            for _ in range(2 if kt < 2 else 1):
                if D_SCORES:
                    hp_n, kt_n = D_SCORES.pop(0)
                    scores_exp(0, hp_n, kt_n, dve=(kt_n % 2 == 1))
            if kt % 4 == 1 and filler:
                f, a = filler.popleft()
                f(*a)
        norm_transp(0, 1)
        for kt in range(NKT):
            av_t(0, 2, kt)
            if kt % 4 == 1 and filler:
                f, a = filler.popleft()
                f(*a)
        norm_transp(0, 2)
        for kt in range(NKT - 1):
            av_t(0, 3, kt)
            if kt % 4 == 1 and filler:
                f, a = filler.popleft()
                f(*a)
        while filler:
            f, a = filler.popleft()
            f(*a)

        # Phase E: steady flattened stream over (hp, kt); each hp's last
        # AV-T and normalize fire after the NEXT hp's first scores so the
        # exp stream never waits on the norm/transpose block. Previous
        # sqt's out-proj and next sqt's Q-proj interleave as PE filler.
        # AV-T at lag-2: scores(kt) waiting on the st-buffer rotation
        # already implies exp(kt-2) completed, so an AV-T emitted two score
        # steps behind never head-of-line blocks the PE queue.
        pend = deque([(0, 3, NKT - 1)])  # phase-D leftover AV-T
        for sqt in range(1, NSQT):
            last = sqt == NSQT - 1
            if not last:
                qproj_load(sqt + 1)
            filler = deque()
            if not last:
                for do in range(NDO):
                    for half in range(2):
                        filler.append((qproj_half, (sqt + 1, do, half)))
            for do in range(DOUT // 512):
                for sc in range(NSQC):
                    filler.append((oproj_chain, (sqt - 1, do, sc)))
            for hp in range(H // 2):
                for kt in range(NKT):
                    scores_exp(sqt, hp, kt, dve=(kt % 2 == 1))
                    pend.append((sqt, hp, kt))
                    if len(pend) > 2:
                        done = pend.popleft()
                        av_t(*done)
                        if done[2] == NKT - 1:
                            norm_transp(done[0], done[1])
                    if kt % 4 == 1 and filler:
                        f, a = filler.popleft()
                        f(*a)
            while filler:
                f, a = filler.popleft()
                f(*a)
        while pend:
            done = pend.popleft()
            av_t(*done)
            if done[2] == NKT - 1 and done != (NSQT - 1, H // 2 - 1, NKT - 1):
                norm_transp(done[0], done[1])
        norm_transp(NSQT - 1, H // 2 - 1, oproj_after_qc=True)

    nc.compile()
    return nc


# ---------------------------------------------------------------------------
# Host-side wrapper: shard across 8 NeuronCores, run SPMD, gather.
# Core c handles batch b = c // 2 and head-group g = c % 2 (8 of 16 heads,
# i.e. columns [g*512, (g+1)*512) of Wq/Wk/Wv and rows of Wo).
# ---------------------------------------------------------------------------

import numpy as np
import ml_dtypes

from concourse.bass_utils import run_bass_kernel_spmd

_NC = None
_BF16 = ml_dtypes.bfloat16


def _get_nc():
    global _NC
    if _NC is None:
        _NC = build_mha_core(S=2048, DIN=1024, DC=512, DOUT=1024, H=8,
                             depth=64, num_devices=8)
    return _NC


def _in_maps(q, k, v, Wq, bq, Wk, bk, Wv, bv, Wo, bo):
    f32 = np.float32
    maps = []
    qb = [np.ascontiguousarray(np.asarray(q[b], dtype=f32).astype(_BF16))
          for b in range(4)]
    kb = [np.ascontiguousarray(np.asarray(k[b], dtype=f32).astype(_BF16))
          for b in range(4)]
    vb = [np.ascontiguousarray(np.asarray(v[b], dtype=f32).astype(_BF16))
          for b in range(4)]
    Wq = np.asarray(Wq, dtype=f32)
    Wk = np.asarray(Wk, dtype=f32)
    Wv = np.asarray(Wv, dtype=f32)
    Wo = np.asarray(Wo, dtype=f32)
    for c in range(8):
        b, g = c // 2, c % 2
        sl = slice(g * 512, (g + 1) * 512)
        maps.append({
            "xq": qb[b],
            "xk": kb[b],
            "xv": vb[b],
            "wq": np.ascontiguousarray(Wq[:, sl].astype(_BF16)),
            "wk": np.ascontiguousarray(Wk[:, sl].astype(_BF16)),
            "wv": np.ascontiguousarray(Wv[:, sl].astype(_BF16)),
            "wo": np.ascontiguousarray(Wo[sl, :].astype(_BF16)),
            "bq": np.ascontiguousarray(bq[sl], dtype=f32),
            "bk": np.ascontiguousarray(bk[sl], dtype=f32),
            "bv": np.ascontiguousarray(bv[sl], dtype=f32),
        })
    return maps


def _gather(results, bo):
    out = np.empty((4, 2048, 1024), dtype=np.float32)
    bo32 = np.asarray(bo, dtype=np.float32)
    for b in range(4):
        out[b] = results[2 * b]["out"] + results[2 * b + 1]["out"] + bo32
    return out


def kernel(q, k, v, Wq, bq, Wk, bk, Wv, bv, Wo, bo, _trace=False):
    nc = _get_nc()
    res = run_bass_kernel_spmd(
        nc, _in_maps(q, k, v, Wq, bq, Wk, bk, Wv, bv, Wo, bo),
        core_ids=list(range(8)), trace=_trace)
    out = _gather(res.results, bo)
    if _trace:
        kernel.last_results = res
    return out


# revision 42
# speedup vs baseline: 1.0207x; 1.0024x over previous
"""Bass/Tile multi-head attention kernel builder for TRN2 (v3).

Per-core problem (core c handles batch b=c//2, head-group g=c%2):
  inputs:  xq, xk, xv [S, DIN] bf16     (batch b slices of q/k/v, host-cast)
           wq, wk, wv [DIN, DC] bf16    (column slice for this head group)
           wo [DC, DOUT] bf16           (row slice)
           bq, bk, bv [DC] f32
  output:  out [S, DOUT] f32  partial:  host sums the two head-group partials
           per batch and adds bo.

Math (per head h of H local heads, depth=64):
  xt   = X^T via DMA-xbar transpose loads          [DIN(p-major blocks), S]
  QT   = (wq_blk.T @ xt) + bq                       [DC, S]  f32r
  KT   = (wk_blk.T @ xt) + bk                       [DC, S]  f32r
  V    = (xt_chunk.T @ wv) + bv (+ ones col)        [S, DC(+1/head)] bf16
  ST   = KT_h.T @ QT_h   (64-partition contraction) [keys, q] per head
  E    = exp(ST * 1/sqrt(depth))  -> bf16           (logits O(10), no max-sub)
  OT   = E_chunk.T @ V_aug_h  (transposed-AV)       [q, depth+1] accum over keys
  O    = OT[:, :depth] / OT[:, depth]  -> bf16      (free-dim normalize)
  OTn  = O^T per head pair (PE transpose)           [DC, S] bf16
  out  = OTn.T @ wo                                 [S, DOUT] f32

The emission order is a hand-rolled software pipeline: the scalar engine
(exp over all S^2 logits) is the throughput floor, so score/exp work is
interleaved into the K/V projection phases and the per-sqt out/Q
projections are spread as PE filler inside the attention kt loop, keeping
both PE and ACT continuously fed.
"""

import math
from collections import deque
from contextlib import ExitStack

import concourse.mybir as mybir
from concourse import bacc
from concourse.masks import make_identity
from concourse.tile import TileContext

F32 = mybir.dt.float32
F32R = mybir.dt.float32r
BF16 = mybir.dt.bfloat16
P = 128
EXP = mybir.ActivationFunctionType.Exp


def build_mha_core(S=2048, DIN=1024, DC=512, DOUT=1024, H=8, depth=64,
                   SQT=512, num_devices=1, ablate="", q_bufs=2, ex_bufs=30,
                   st_bufs=4, xt_bufs=2):
    ablate = set(ablate.split(",")) if ablate else set()
    assert DC == H * depth and DC % P == 0 and DIN % P == 0 and S % SQT == 0
    NKT = S // P          # key chunks of 128
    NDIN = DIN // P       # input-dim k-tiles
    NDO = DC // P         # d_core blocks
    NSQT = S // SQT       # attention q tiles
    NSQC = SQT // P       # 128-query chunks per sqt
    scale = 1.0 / float(depth) ** 0.5

    nc = bacc.Bacc("TRN2", target_bir_lowering=False, debug=False,
                   num_devices=num_devices)
    xq = nc.dram_tensor("xq", [S, DIN], BF16, kind="ExternalInput")
    xk = nc.dram_tensor("xk", [S, DIN], BF16, kind="ExternalInput")
    xv = nc.dram_tensor("xv", [S, DIN], BF16, kind="ExternalInput")
    wq = nc.dram_tensor("wq", [DIN, DC], BF16, kind="ExternalInput")
    wk = nc.dram_tensor("wk", [DIN, DC], BF16, kind="ExternalInput")
    wv = nc.dram_tensor("wv", [DIN, DC], BF16, kind="ExternalInput")
    wo = nc.dram_tensor("wo", [DC, DOUT], BF16, kind="ExternalInput")
    bq = nc.dram_tensor("bq", [DC], F32, kind="ExternalInput")
    bk = nc.dram_tensor("bk", [DC], F32, kind="ExternalInput")
    bv = nc.dram_tensor("bv", [DC], F32, kind="ExternalInput")
    out = nc.dram_tensor("out", [S, DOUT], F32, kind="ExternalOutput")

    with TileContext(nc) as tc, ExitStack() as ctx:
        const = ctx.enter_context(tc.tile_pool(name="const", bufs=1))
        wpool = ctx.enter_context(tc.tile_pool(name="wpool", bufs=1))
        kvpool = ctx.enter_context(tc.tile_pool(name="kv", bufs=1))
        xtkv = ctx.enter_context(tc.tile_pool(name="xtkv", bufs=xt_bufs))
        xtq = ctx.enter_context(tc.tile_pool(name="xtq", bufs=xt_bufs))
        qpool = ctx.enter_context(tc.tile_pool(name="qp", bufs=q_bufs))
        expool = ctx.enter_context(tc.tile_pool(name="ex", bufs=ex_bufs))
        opool = ctx.enter_context(tc.tile_pool(name="op", bufs=2))
        misc = ctx.enter_context(tc.tile_pool(name="misc", bufs=2))
        ps_st = ctx.enter_context(tc.tile_pool(name="ps_st", bufs=st_bufs,
                                               space="PSUM"))
        ps_ot = ctx.enter_context(tc.tile_pool(name="ps_ot", bufs=1,
                                               space="PSUM"))
        ps_gen = ctx.enter_context(tc.tile_pool(name="ps_gen", bufs=2,
                                                space="PSUM"))

        ident = const.tile([P, P], BF16)
        make_identity(nc, ident)
        # warm the Exp activation table while the first DMAs are in flight
        warm = const.tile([1, 2], F32)
        nc.vector.memset(warm[:], 0.0)
        nc.scalar.activation(warm[0:1, 0:1], warm[0:1, 1:2], EXP)

        # ---- weights: direct bf16 DMA loads, no staging ----
        def load_weight(dram, kdim, ndim, name, split=False):
            w = wpool.tile([P, kdim // P, ndim], BF16, name=name)
            if split:
                # first output block loads first: unblocks the do=0 chain
                nc.sync.dma_start(
                    w[:, :, 0:P],
                    dram[:, 0:P].rearrange("(o p) n -> p o n", p=P))
                nc.sync.dma_start(
                    w[:, :, P:ndim],
                    dram[:, P:].rearrange("(o p) n -> p o n", p=P))
            else:
                nc.sync.dma_start(
                    w[:], dram[:, :].rearrange("(o p) n -> p o n", p=P))
            return w

        KT = kvpool.tile([P, NDO, S], F32R)
        V = kvpool.tile([P, NKT, H, depth + 1], BF16)
        nc.vector.memset(V[:, :, :, depth:depth + 1], 1.0)

        # ---------------- emitters ----------------
        def kproj_half(xt, st_i, do, half):
            ps = ps_gen.tile([P, 256], F32, tag="gen", name="pskh")
            for kt in range(NDIN):
                nc.tensor.matmul(
                    ps[:], wk_sb[:, kt, do * P:(do + 1) * P],
                    xt[:, kt, half * 256:(half + 1) * 256],
                    start=(kt == 0), stop=(kt == NDIN - 1))
            nc.vector.tensor_scalar_add(
                KT[:, do, st_i * 512 + half * 256:st_i * 512 + half * 256
                   + 256], ps[:], bk_sb[:, do:do + 1])

        def kproj_do0(st_i, split=False):
            xt = xtkv.tile([P, NDIN, 512], BF16, tag="xt", name="xtk")
            rows = xk[st_i * 512:(st_i + 1) * 512, :]
            if split:
                # row-split so the first scores unblock after half a chunk
                nc.sync.dma_start_transpose(xt[:, :, 0:256], rows[0:256, :])
                kproj_half(xt, st_i, 0, 0)
                scores_exp(0, 0, 4 * st_i)
                scores_exp(0, 0, 4 * st_i + 1)
                nc.sync.dma_start_transpose(xt[:, :, 256:512],
                                            rows[256:512, :])
                kproj_half(xt, st_i, 0, 1)
            else:
                nc.sync.dma_start_transpose(xt[:], rows)
                kproj_rest(xt, st_i, dos=(0,))
            return xt

        def kproj_rest(xt, st_i, dos=(1, 2, 3)):
            for do in dos:
                ps = ps_gen.tile([P, 512], F32, tag="gen", name="psk")
                for kt in range(NDIN):
                    nc.tensor.matmul(
                        ps[:], wk_sb[:, kt, do * P:(do + 1) * P], xt[:, kt, :],
                        start=(kt == 0), stop=(kt == NDIN - 1))
                nc.vector.tensor_scalar_add(
                    KT[:, do, st_i * 512:(st_i + 1) * 512], ps[:],
                    bk_sb[:, do:do + 1])

        def vproj_chunk(st_i, after_sc=None):
            xt = xtkv.tile([P, NDIN, 512], BF16, tag="xt", name="xtv")
            nc.sync.dma_start_transpose(
                xt[:], xv[st_i * 512:(st_i + 1) * 512, :])
            for sc in range(4):
                ps = ps_gen.tile([P, 512], F32, tag="gen", name="psv")
                for kt in range(NDIN):
                    nc.tensor.matmul(
                        ps[:], xt[:, kt, sc * P:(sc + 1) * P], wv_sb[:, kt, :],
                        start=(kt == 0), stop=(kt == NDIN - 1))
                chunk = st_i * 4 + sc
                nc.vector.tensor_tensor(
                    V[:, chunk, :, 0:depth],
                    ps[:].rearrange("p (h d) -> p h d", h=H),
                    bv_bc[:].rearrange("p (h d) -> p h d", h=H),
                    mybir.AluOpType.add)
                if after_sc is not None:
                    after_sc(chunk)

        QTs = {}

        def qproj_load(sqt, split_first=False):
            xt = xtq.tile([P, NDIN, SQT], BF16, tag="xt", name="xtq")
            rows = xq[sqt * SQT:(sqt + 1) * SQT, :]
            if split_first:
                # load the first k-tile separately so chain kt=0 can start
                # before the bulk of the transpose-load finishes
                nc.sync.dma_start_transpose(xt[:, 0:1, :], rows[:, 0:P])
                nc.sync.dma_start_transpose(xt[:, 1:NDIN, :], rows[:, P:])
            else:
                nc.sync.dma_start_transpose(xt[:], rows)
            QTs[sqt] = (qpool.tile([P, NDO, SQT], F32R, tag="qt", name="qt"),
                        xt)

        def qproj_chain(sqt, do):
            QT, xt = QTs[sqt]
            ps = ps_gen.tile([P, 512], F32, tag="gen", name="psq")
            for kt in range(NDIN):
                nc.tensor.matmul(
                    ps[:], wq_sb[:, kt, do * P:(do + 1) * P], xt[:, kt, :],
                    start=(kt == 0), stop=(kt == NDIN - 1))
            nc.vector.tensor_scalar_add(QT[:, do, :], ps[:],
                                        bq_sb[:, do:do + 1])

        def qproj_half(sqt, do, half):
            # finer-grained filler: half the free dim per chain
            QT, xt = QTs[sqt]
            ps = ps_gen.tile([P, 256], F32, tag="gen", name="psqh")
            for kt in range(NDIN):
                nc.tensor.matmul(
                    ps[:], wq_sb[:, kt, do * P:(do + 1) * P],
                    xt[:, kt, half * 256:(half + 1) * 256],
                    start=(kt == 0), stop=(kt == NDIN - 1))
            nc.vector.tensor_scalar_add(
                QT[:, do, half * 256:(half + 1) * 256], ps[:],
                bq_sb[:, do:do + 1])

        ex_map = {}
        ot_map = {}
        OTns = {}

        # Schraudolph bit-trick exp for the DVE: exp(s*x) ~=
        # bitcast_bf16(int16(A*x + B)) with A = 128*s/ln2, B = 127*128 - c.
        # ~+-3% per weight, self-consistent through the softmax denominator
        # (it sums the same approximated values). Used on a fraction of key
        # tiles to offload the scalar engine, which is the throughput floor.
        EXPA = 128.0 * scale / math.log(2.0)
        EXPB = 127.0 * 128.0 - 7.42

        def scores_exp(sqt, hp, kt, dve=False):
            # per-head one-bank st tiles: a 4-slot rotation in the same 4
            # PSUM banks doubles the scores->exp pipeline elasticity, and
            # per-head exp ops allow a finer ACT/DVE split.
            QT = QTs[sqt][0]
            ex = expool.tile([P, 2, 512], BF16, tag="ex", name="ex")
            for hi, h in enumerate((2 * hp, 2 * hp + 1)):
                st = ps_st.tile([P, 512], F32, name="st")
                p0 = (h % 2) * 64
                nc.tensor.matmul(
                    st[:],
                    KT[p0:p0 + 64, hp, kt * P:(kt + 1) * P],
                    QT[p0:p0 + 64, hp, :],
                    start=True, stop=True)
                on_dve = dve and (hi == 1 or kt % 4 == 3)
                if on_dve:
                    nc.vector.tensor_scalar(
                        ex[:, hi, :].bitcast(mybir.dt.int16), st[:],
                        EXPA, EXPB,
                        mybir.AluOpType.mult, mybir.AluOpType.add)
                else:
                    nc.scalar.activation(ex[:, hi, :], st[:], EXP,
                                         scale=scale)
            ex_map[(sqt, hp, kt)] = ex

        def av_t(sqt, hp, kt):
            if kt == 0:
                ot_map[(sqt, hp)] = [
                    ps_ot.tile([P, NSQC, P], F32, name=f"ot{i}")
                    for i in range(2)]
            ot_ps = ot_map[(sqt, hp)]
            ex = ex_map.pop((sqt, hp, kt))
            for hi, h in enumerate((2 * hp, 2 * hp + 1)):
                for qc in range(NSQC):
                    # start zeroes the whole 2KB PSUM zero-region (bank), so
                    # only the very first matmul into each head's bank starts
                    # the group; all four qc regions then accumulate onto
                    # zeros.
                    nc.tensor.matmul(
                        ot_ps[hi][:, qc, 0:depth + 1],
                        ex[:, hi, qc * P:(qc + 1) * P],
                        V[:, kt, h, :],
                        start=(kt == 0 and qc == 0),
                        stop=(kt == NKT - 1 and qc == NSQC - 1),
                        skip_group_check=True)

        def norm_transp(sqt, hp, oproj_after_qc=False):
            if hp == 0:
                OTns[sqt] = opool.tile([P, NDO, SQT], BF16, tag="otn",
                                       name="otn")
            OTn = OTns[sqt]
            ot_ps = ot_map.pop((sqt, hp))
            O_sb = opool.tile([P, NSQC, 2, depth], BF16, tag="osb",
                              name="osb")
            for hi in range(2):
                rec = misc.tile([P, NSQC, 1], F32, tag="rec", name="rec")
                nc.vector.reciprocal(rec[:],
                                     ot_ps[hi][:, :, depth:depth + 1])
                nc.vector.tensor_tensor(
                    O_sb[:, :, hi, :], ot_ps[hi][:, :, 0:depth],
                    rec[:].to_broadcast((P, NSQC, depth)),
                    mybir.AluOpType.mult)
            for qc in range(NSQC):
                tp = ps_gen.tile([P, P], BF16, tag="gen", name="tp")
                nc.tensor.transpose(tp[:], O_sb[:, qc, :, :], ident[:])
                nc.vector.tensor_copy(
                    OTn[:, hp, qc * P:(qc + 1) * P], tp[:])
                if oproj_after_qc:
                    for do in range(DOUT // 512):
                        oproj_chain(sqt, do, qc, direct=True)

        def oproj_chain(sqt, do, sc, direct=False):
            OTn = OTns[sqt]
            ps = ps_gen.tile([P, 512], F32, tag="gen", name="pso")
            for hh in range(NDO):
                nc.tensor.matmul(
                    ps[:], OTn[:, hh, sc * P:(sc + 1) * P],
                    wo_sb[:, hh, do * 512:(do + 1) * 512],
                    start=(hh == 0), stop=(hh == NDO - 1))
            r0 = sqt * SQT + sc * P
            osb = misc.tile([P, 512], F32, tag="osb2", name="osb2")
            nc.vector.tensor_copy(osb[:], ps[:])
            nc.sync.dma_start(out[r0:r0 + P, do * 512:(do + 1) * 512],
                              osb[:])

        # ---------------- schedule ----------------
        # Loads ordered so the first scores+exp are unblocked ASAP: the
        # critical DMA chain is xtq-T, wq, wk, xtk-half0-T.
        qproj_load(0)
        wq_sb = load_weight(wq, DIN, DC, "wq_sb")
        bq_sb = const.tile([P, NDO], F32)
        nc.sync.dma_start(bq_sb[:], bq[:].rearrange("(o p) -> p o", p=P))
        qproj_chain(0, 0)
        wk_sb = load_weight(wk, DIN, DC, "wk_sb")
        bk_sb = const.tile([P, NDO], F32)
        nc.sync.dma_start(bk_sb[:], bk[:].rearrange("(o p) -> p o", p=P))

        # Phase B: K-proj, with sqt0 hp0 (and first hp1) scores+exp fused in.
        for st_i in range(4):
            xt = kproj_do0(st_i, split=(st_i == 0))
            if st_i == 0:
                scores_exp(0, 0, 2)
                scores_exp(0, 0, 3)
                kproj_rest(xt, 0, dos=(1,))
                for do in range(1, NDO):
                    qproj_chain(0, do)
                kproj_rest(xt, 0, dos=(2, 3))
            else:
                for kt in range(4 * st_i, 4 * st_i + 4):
                    scores_exp(0, 0, kt)
                kproj_rest(xt, st_i)
            if st_i == 3:
                for kt in range(0, 8):
                    scores_exp(0, 1, kt)
        # Phase C: V-proj groups; consume hp0 via AV-T as each V chunk
        # lands, keep the exp stream fed with hp1/hp2 scores.
        wv_sb = load_weight(wv, DIN, DC, "wv_sb")
        bv_st = const.tile([1, DC], F32)
        nc.sync.dma_start(bv_st[0:1, :], bv[:][None, :])
        bv_bc = const.tile([P, DC], F32)
        nc.gpsimd.partition_broadcast(bv_bc[:], bv_st[0:1, :])
        wo_sb = load_weight(wo, DC, DOUT, "wo_sb")
        C_SCORES = [(1, kt) for kt in range(8, 16)] + \
                   [(2, kt) for kt in range(0, 14)]
        for st_i in range(4):
            def consume(chunk):
                av_t(0, 0, chunk)
                npop = 2 if chunk < 8 else 1
                for _ in range(npop):
                    if C_SCORES:
                        hp_n, kt = C_SCORES.pop(0)
                        scores_exp(0, hp_n, kt)
            vproj_chunk(st_i, after_sc=consume)
        # Phase D: finish sqt0 (hp1..hp3), qproj(1) as filler.
        qproj_load(1)
        filler = deque()
        for do in range(NDO):
            filler.append((qproj_chain, (1, do)))
        norm_transp(0, 0)
        D_SCORES = [(2, 14), (2, 15)] + [(3, kt) for kt in range(NKT)]
        for kt in range(NKT):
            av_t(0, 1, kt)
            for _ in range(2 if kt < 2 else 1):
                if D_SCORES:
                    hp_n, kt_n = D_SCORES.pop(0)
                    scores_exp(0, hp_n, kt_n, dve=(kt_n % 2 == 1))
            if kt % 4 == 1 and filler:
                f, a = filler.popleft()
                f(*a)
        norm_transp(0, 1)
        for kt in range(NKT):
            av_t(0, 2, kt)
            if kt % 4 == 1 and filler:
                f, a = filler.popleft()
                f(*a)
        norm_transp(0, 2)
        for kt in range(NKT - 1):
            av_t(0, 3, kt)
            if kt % 4 == 1 and filler:
                f, a = filler.popleft()
                f(*a)
        while filler:
            f, a = filler.popleft()
            f(*a)

        # Phase E: steady flattened stream over (hp, kt); each hp's last
        # AV-T and normalize fire after the NEXT hp's first scores so the
        # exp stream never waits on the norm/transpose block. Previous
        # sqt's out-proj and next sqt's Q-proj interleave as PE filler.
        # AV-T at lag-2: scores(kt) waiting on the st-buffer rotation
        # already implies exp(kt-2) completed, so an AV-T emitted two score
        # steps behind never head-of-line blocks the PE queue.
        pend = deque([(0, 3, NKT - 1)])  # phase-D leftover AV-T
        for sqt in range(1, NSQT):
            last = sqt == NSQT - 1
            if not last:
                qproj_load(sqt + 1)
            filler = deque()
            if not last:
                for do in range(NDO):
                    for half in range(2):
                        filler.append((qproj_half, (sqt + 1, do, half)))
            for do in range(DOUT // 512):
                for sc in range(NSQC):
                    filler.append((oproj_chain, (sqt - 1, do, sc)))
            for hp in range(H // 2):
                for kt in range(NKT):
                    scores_exp(sqt, hp, kt, dve=(kt % 2 == 1))
                    pend.append((sqt, hp, kt))
                    if len(pend) > 2:
                        done = pend.popleft()
                        av_t(*done)
                        if done[2] == NKT - 1:
                            norm_transp(done[0], done[1])
                    if kt % 4 == 1 and filler:
                        f, a = filler.popleft()
                        f(*a)
            while filler:
                f, a = filler.popleft()
                f(*a)
        while pend:
            done = pend.popleft()
            av_t(*done)
            if done[2] == NKT - 1 and done != (NSQT - 1, H // 2 - 1, NKT - 1):
                norm_transp(done[0], done[1])
        norm_transp(NSQT - 1, H // 2 - 1, oproj_after_qc=True)

    nc.compile()
    return nc


# ---------------------------------------------------------------------------
# Host-side wrapper: shard across 8 NeuronCores, run SPMD, gather.
# Core c handles batch b = c // 2 and head-group g = c % 2 (8 of 16 heads,
# i.e. columns [g*512, (g+1)*512) of Wq/Wk/Wv and rows of Wo).
# ---------------------------------------------------------------------------

import numpy as np
import ml_dtypes

from concourse.bass_utils import run_bass_kernel_spmd

_NC = None
_BF16 = ml_dtypes.bfloat16


def _get_nc():
    global _NC
    if _NC is None:
        _NC = build_mha_core(S=2048, DIN=1024, DC=512, DOUT=1024, H=8,
                             depth=64, num_devices=8)
    return _NC


def _in_maps(q, k, v, Wq, bq, Wk, bk, Wv, bv, Wo, bo):
    f32 = np.float32
    maps = []
    qb = [np.ascontiguousarray(np.asarray(q[b], dtype=f32).astype(_BF16))
          for b in range(4)]
    kb = [np.ascontiguousarray(np.asarray(k[b], dtype=f32).astype(_BF16))
          for b in range(4)]
    vb = [np.ascontiguousarray(np.asarray(v[b], dtype=f32).astype(_BF16))
          for b in range(4)]
    Wq = np.asarray(Wq, dtype=f32)
    Wk = np.asarray(Wk, dtype=f32)
    Wv = np.asarray(Wv, dtype=f32)
    Wo = np.asarray(Wo, dtype=f32)
    for c in range(8):
        b, g = c // 2, c % 2
        sl = slice(g * 512, (g + 1) * 512)
        maps.append({
            "xq": qb[b],
            "xk": kb[b],
            "xv": vb[b],
            "wq": np.ascontiguousarray(Wq[:, sl].astype(_BF16)),
            "wk": np.ascontiguousarray(Wk[:, sl].astype(_BF16)),
            "wv": np.ascontiguousarray(Wv[:, sl].astype(_BF16)),
            "wo": np.ascontiguousarray(Wo[sl, :].astype(_BF16)),
            "bq": np.ascontiguousarray(bq[sl], dtype=f32),
            "bk": np.ascontiguousarray(bk[sl], dtype=f32),
            "bv": np.ascontiguousarray(bv[sl], dtype=f32),
        })
    return maps


def _gather(results, bo):
    out = np.empty((4, 2048, 1024), dtype=np.float32)
    bo32 = np.asarray(bo, dtype=np.float32)
    for b in range(4):
        out[b] = results[2 * b]["out"] + results[2 * b + 1]["out"] + bo32
    return out


def kernel(q, k, v, Wq, bq, Wk, bk, Wv, bv, Wo, bo, _trace=False):
    nc = _get_nc()
    res = run_bass_kernel_spmd(
        nc, _in_maps(q, k, v, Wq, bq, Wk, bk, Wv, bv, Wo, bo),
        core_ids=list(range(8)), trace=_trace)
    out = _gather(res.results, bo)
    if _trace:
        kernel.last_results = res
    return out


# revision 43
# speedup vs baseline: 1.0241x; 1.0033x over previous
"""Bass/Tile multi-head attention kernel builder for TRN2 (v3).

Per-core problem (core c handles batch b=c//2, head-group g=c%2):
  inputs:  xq, xk, xv [S, DIN] bf16     (batch b slices of q/k/v, host-cast)
           wq, wk, wv [DIN, DC] bf16    (column slice for this head group)
           wo [DC, DOUT] bf16           (row slice)
           bq, bk, bv [DC] f32
  output:  out [S, DOUT] f32  partial:  host sums the two head-group partials
           per batch and adds bo.

Math (per head h of H local heads, depth=64):
  xt   = X^T via DMA-xbar transpose loads          [DIN(p-major blocks), S]
  QT   = (wq_blk.T @ xt) + bq                       [DC, S]  f32r
  KT   = (wk_blk.T @ xt) + bk                       [DC, S]  f32r
  V    = (xt_chunk.T @ wv) + bv (+ ones col)        [S, DC(+1/head)] bf16
  ST   = KT_h.T @ QT_h   (64-partition contraction) [keys, q] per head
  E    = exp(ST * 1/sqrt(depth))  -> bf16           (logits O(10), no max-sub)
  OT   = E_chunk.T @ V_aug_h  (transposed-AV)       [q, depth+1] accum over keys
  O    = OT[:, :depth] / OT[:, depth]  -> bf16      (free-dim normalize)
  OTn  = O^T per head pair (PE transpose)           [DC, S] bf16
  out  = OTn.T @ wo                                 [S, DOUT] f32

The emission order is a hand-rolled software pipeline: the scalar engine
(exp over all S^2 logits) is the throughput floor, so score/exp work is
interleaved into the K/V projection phases and the per-sqt out/Q
projections are spread as PE filler inside the attention kt loop, keeping
both PE and ACT continuously fed.
"""

import math
from collections import deque
from contextlib import ExitStack

import concourse.mybir as mybir
from concourse import bacc
from concourse.masks import make_identity
from concourse.tile import TileContext

F32 = mybir.dt.float32
F32R = mybir.dt.float32r
BF16 = mybir.dt.bfloat16
P = 128
EXP = mybir.ActivationFunctionType.Exp


def build_mha_core(S=2048, DIN=1024, DC=512, DOUT=1024, H=8, depth=64,
                   SQT=512, num_devices=1, ablate="", q_bufs=2, ex_bufs=30,
                   st_bufs=4, xt_bufs=2):
    ablate = set(ablate.split(",")) if ablate else set()
    assert DC == H * depth and DC % P == 0 and DIN % P == 0 and S % SQT == 0
    NKT = S // P          # key chunks of 128
    NDIN = DIN // P       # input-dim k-tiles
    NDO = DC // P         # d_core blocks
    NSQT = S // SQT       # attention q tiles
    NSQC = SQT // P       # 128-query chunks per sqt
    scale = 1.0 / float(depth) ** 0.5

    nc = bacc.Bacc("TRN2", target_bir_lowering=False, debug=False,
                   num_devices=num_devices)
    xq = nc.dram_tensor("xq", [S, DIN], BF16, kind="ExternalInput")
    xk = nc.dram_tensor("xk", [S, DIN], BF16, kind="ExternalInput")
    xv = nc.dram_tensor("xv", [S, DIN], BF16, kind="ExternalInput")
    wq = nc.dram_tensor("wq", [DIN, DC], BF16, kind="ExternalInput")
    wk = nc.dram_tensor("wk", [DIN, DC], BF16, kind="ExternalInput")
    wv = nc.dram_tensor("wv", [DIN, DC], BF16, kind="ExternalInput")
    wo = nc.dram_tensor("wo", [DC, DOUT], BF16, kind="ExternalInput")
    bq = nc.dram_tensor("bq", [DC], F32, kind="ExternalInput")
    bk = nc.dram_tensor("bk", [DC], F32, kind="ExternalInput")
    bv = nc.dram_tensor("bv", [DC], F32, kind="ExternalInput")
    out = nc.dram_tensor("out", [S, DOUT], F32, kind="ExternalOutput")

    with TileContext(nc) as tc, ExitStack() as ctx:
        const = ctx.enter_context(tc.tile_pool(name="const", bufs=1))
        wpool = ctx.enter_context(tc.tile_pool(name="wpool", bufs=1))
        kvpool = ctx.enter_context(tc.tile_pool(name="kv", bufs=1))
        xtkv = ctx.enter_context(tc.tile_pool(name="xtkv", bufs=xt_bufs))
        xtq = ctx.enter_context(tc.tile_pool(name="xtq", bufs=xt_bufs))
        qpool = ctx.enter_context(tc.tile_pool(name="qp", bufs=q_bufs))
        expool = ctx.enter_context(tc.tile_pool(name="ex", bufs=ex_bufs))
        opool = ctx.enter_context(tc.tile_pool(name="op", bufs=2))
        misc = ctx.enter_context(tc.tile_pool(name="misc", bufs=2))
        ps_st = ctx.enter_context(tc.tile_pool(name="ps_st", bufs=st_bufs,
                                               space="PSUM"))
        ps_ot = ctx.enter_context(tc.tile_pool(name="ps_ot", bufs=1,
                                               space="PSUM"))
        ps_gen = ctx.enter_context(tc.tile_pool(name="ps_gen", bufs=2,
                                                space="PSUM"))

        ident = const.tile([P, P], BF16)
        make_identity(nc, ident)
        # warm the Exp activation table while the first DMAs are in flight
        warm = const.tile([1, 2], F32)
        nc.vector.memset(warm[:], 0.0)
        nc.scalar.activation(warm[0:1, 0:1], warm[0:1, 1:2], EXP)

        # ---- weights: direct bf16 DMA loads, no staging ----
        def load_weight(dram, kdim, ndim, name, split=False):
            w = wpool.tile([P, kdim // P, ndim], BF16, name=name)
            if split:
                # first output block loads first: unblocks the do=0 chain
                nc.sync.dma_start(
                    w[:, :, 0:P],
                    dram[:, 0:P].rearrange("(o p) n -> p o n", p=P))
                nc.sync.dma_start(
                    w[:, :, P:ndim],
                    dram[:, P:].rearrange("(o p) n -> p o n", p=P))
            else:
                nc.sync.dma_start(
                    w[:], dram[:, :].rearrange("(o p) n -> p o n", p=P))
            return w

        KT = kvpool.tile([P, NDO, S], F32R)
        V = kvpool.tile([P, NKT, H, depth + 1], BF16)
        nc.vector.memset(V[:, :, :, depth:depth + 1], 1.0)

        # ---------------- emitters ----------------
        def kproj_half(xt, st_i, do, half):
            ps = ps_gen.tile([P, 256], F32, tag="gen", name="pskh")
            for kt in range(NDIN):
                nc.tensor.matmul(
                    ps[:], wk_sb[:, kt, do * P:(do + 1) * P],
                    xt[:, kt, half * 256:(half + 1) * 256],
                    start=(kt == 0), stop=(kt == NDIN - 1))
            nc.vector.tensor_scalar_add(
                KT[:, do, st_i * 512 + half * 256:st_i * 512 + half * 256
                   + 256], ps[:], bk_sb[:, do:do + 1])

        def kproj_do0(st_i, split=False):
            xt = xtkv.tile([P, NDIN, 512], BF16, tag="xt", name="xtk")
            rows = xk[st_i * 512:(st_i + 1) * 512, :]
            if split:
                # row-split so the first scores unblock after half a chunk
                nc.sync.dma_start_transpose(xt[:, :, 0:256], rows[0:256, :])
                kproj_half(xt, st_i, 0, 0)
                scores_exp(0, 0, 4 * st_i)
                scores_exp(0, 0, 4 * st_i + 1)
                nc.sync.dma_start_transpose(xt[:, :, 256:512],
                                            rows[256:512, :])
                kproj_half(xt, st_i, 0, 1)
            else:
                nc.sync.dma_start_transpose(xt[:], rows)
                kproj_rest(xt, st_i, dos=(0,))
            return xt

        def kproj_rest(xt, st_i, dos=(1, 2, 3)):
            for do in dos:
                ps = ps_gen.tile([P, 512], F32, tag="gen", name="psk")
                for kt in range(NDIN):
                    nc.tensor.matmul(
                        ps[:], wk_sb[:, kt, do * P:(do + 1) * P], xt[:, kt, :],
                        start=(kt == 0), stop=(kt == NDIN - 1))
                nc.vector.tensor_scalar_add(
                    KT[:, do, st_i * 512:(st_i + 1) * 512], ps[:],
                    bk_sb[:, do:do + 1])

        def vproj_chunk(st_i, after_sc=None):
            xt = xtkv.tile([P, NDIN, 512], BF16, tag="xt", name="xtv")
            nc.sync.dma_start_transpose(
                xt[:], xv[st_i * 512:(st_i + 1) * 512, :])
            for sc in range(4):
                ps = ps_gen.tile([P, 512], F32, tag="gen", name="psv")
                for kt in range(NDIN):
                    nc.tensor.matmul(
                        ps[:], xt[:, kt, sc * P:(sc + 1) * P], wv_sb[:, kt, :],
                        start=(kt == 0), stop=(kt == NDIN - 1))
                chunk = st_i * 4 + sc
                nc.vector.tensor_tensor(
                    V[:, chunk, :, 0:depth],
                    ps[:].rearrange("p (h d) -> p h d", h=H),
                    bv_bc[:].rearrange("p (h d) -> p h d", h=H),
                    mybir.AluOpType.add)
                if after_sc is not None:
                    after_sc(chunk)

        QTs = {}

        def qproj_load(sqt, split_first=False):
            xt = xtq.tile([P, NDIN, SQT], BF16, tag="xt", name="xtq")
            rows = xq[sqt * SQT:(sqt + 1) * SQT, :]
            if split_first:
                # load the first k-tile separately so chain kt=0 can start
                # before the bulk of the transpose-load finishes
                nc.sync.dma_start_transpose(xt[:, 0:1, :], rows[:, 0:P])
                nc.sync.dma_start_transpose(xt[:, 1:NDIN, :], rows[:, P:])
            else:
                nc.sync.dma_start_transpose(xt[:], rows)
            QTs[sqt] = (qpool.tile([P, NDO, SQT], F32R, tag="qt", name="qt"),
                        xt)

        def qproj_chain(sqt, do):
            QT, xt = QTs[sqt]
            ps = ps_gen.tile([P, 512], F32, tag="gen", name="psq")
            for kt in range(NDIN):
                nc.tensor.matmul(
                    ps[:], wq_sb[:, kt, do * P:(do + 1) * P], xt[:, kt, :],
                    start=(kt == 0), stop=(kt == NDIN - 1))
            nc.vector.tensor_scalar_add(QT[:, do, :], ps[:],
                                        bq_sb[:, do:do + 1])

        def qproj_half(sqt, do, half):
            # finer-grained filler: half the free dim per chain
            QT, xt = QTs[sqt]
            ps = ps_gen.tile([P, 256], F32, tag="gen", name="psqh")
            for kt in range(NDIN):
                nc.tensor.matmul(
                    ps[:], wq_sb[:, kt, do * P:(do + 1) * P],
                    xt[:, kt, half * 256:(half + 1) * 256],
                    start=(kt == 0), stop=(kt == NDIN - 1))
            nc.vector.tensor_scalar_add(
                QT[:, do, half * 256:(half + 1) * 256], ps[:],
                bq_sb[:, do:do + 1])

        ex_map = {}
        ot_map = {}
        OTns = {}

        # Schraudolph bit-trick exp for the DVE: exp(s*x) ~=
        # bitcast_bf16(int16(A*x + B)) with A = 128*s/ln2, B = 127*128 - c.
        # ~+-3% per weight, self-consistent through the softmax denominator
        # (it sums the same approximated values). Used on a fraction of key
        # tiles to offload the scalar engine, which is the throughput floor.
        EXPA = 128.0 * scale / math.log(2.0)
        EXPB = 127.0 * 128.0 - 7.42

        def scores_exp(sqt, hp, kt, dve=False):
            # per-head one-bank st tiles: a 4-slot rotation in the same 4
            # PSUM banks doubles the scores->exp pipeline elasticity, and
            # per-head exp ops allow a finer ACT/DVE split.
            QT = QTs[sqt][0]
            ex = expool.tile([P, 2, 512], BF16, tag="ex", name="ex")
            for hi, h in enumerate((2 * hp, 2 * hp + 1)):
                st = ps_st.tile([P, 512], F32, name="st")
                p0 = (h % 2) * 64
                nc.tensor.matmul(
                    st[:],
                    KT[p0:p0 + 64, hp, kt * P:(kt + 1) * P],
                    QT[p0:p0 + 64, hp, :],
                    start=True, stop=True)
                on_dve = dve and (hi == 1 or kt % 4 == 3)
                if on_dve:
                    nc.vector.tensor_scalar(
                        ex[:, hi, :].bitcast(mybir.dt.int16), st[:],
                        EXPA, EXPB,
                        mybir.AluOpType.mult, mybir.AluOpType.add)
                else:
                    nc.scalar.activation(ex[:, hi, :], st[:], EXP,
                                         scale=scale)
            ex_map[(sqt, hp, kt)] = ex

        def av_t(sqt, hp, kt):
            if kt == 0:
                ot_map[(sqt, hp)] = [
                    ps_ot.tile([P, NSQC, P], F32, name=f"ot{i}")
                    for i in range(2)]
            ot_ps = ot_map[(sqt, hp)]
            ex = ex_map.pop((sqt, hp, kt))
            for hi, h in enumerate((2 * hp, 2 * hp + 1)):
                for qc in range(NSQC):
                    # start zeroes the whole 2KB PSUM zero-region (bank), so
                    # only the very first matmul into each head's bank starts
                    # the group; all four qc regions then accumulate onto
                    # zeros.
                    nc.tensor.matmul(
                        ot_ps[hi][:, qc, 0:depth + 1],
                        ex[:, hi, qc * P:(qc + 1) * P],
                        V[:, kt, h, :],
                        start=(kt == 0 and qc == 0),
                        stop=(kt == NKT - 1 and qc == NSQC - 1),
                        skip_group_check=True)

        def norm_transp(sqt, hp, oproj_after_qc=False):
            if hp == 0:
                OTns[sqt] = opool.tile([P, NDO, SQT], BF16, tag="otn",
                                       name="otn")
            OTn = OTns[sqt]
            ot_ps = ot_map.pop((sqt, hp))
            O_sb = opool.tile([P, NSQC, 2, depth], BF16, tag="osb",
                              name="osb")
            for hi in range(2):
                rec = misc.tile([P, NSQC, 1], F32, tag="rec", name="rec")
                nc.vector.reciprocal(rec[:],
                                     ot_ps[hi][:, :, depth:depth + 1])
                nc.vector.tensor_tensor(
                    O_sb[:, :, hi, :], ot_ps[hi][:, :, 0:depth],
                    rec[:].to_broadcast((P, NSQC, depth)),
                    mybir.AluOpType.mult)
            # all four 128-wide transposes accumulate into one PSUM bank
            # (start zeroes it, later ones add onto zeros), then a single
            # 512-wide copy moves the whole head-pair row into OTn.
            tp = ps_gen.tile([P, 4 * P], BF16, tag="gen", name="tp")
            for qc in range(NSQC):
                nc.tensor.matmul(
                    tp[:, qc * P:(qc + 1) * P], O_sb[:, qc, :, :], ident[:],
                    is_transpose=True, start=(qc == 0),
                    stop=(qc == NSQC - 1), skip_group_check=True)
            nc.vector.tensor_copy(OTn[:, hp, :], tp[:])
            if oproj_after_qc:
                for do in range(DOUT // 512):
                    for sc in range(NSQC):
                        oproj_chain(sqt, do, sc)

        def oproj_chain(sqt, do, sc, direct=False):
            OTn = OTns[sqt]
            ps = ps_gen.tile([P, 512], F32, tag="gen", name="pso")
            for hh in range(NDO):
                nc.tensor.matmul(
                    ps[:], OTn[:, hh, sc * P:(sc + 1) * P],
                    wo_sb[:, hh, do * 512:(do + 1) * 512],
                    start=(hh == 0), stop=(hh == NDO - 1))
            r0 = sqt * SQT + sc * P
            osb = misc.tile([P, 512], F32, tag="osb2", name="osb2")
            nc.vector.tensor_copy(osb[:], ps[:])
            nc.sync.dma_start(out[r0:r0 + P, do * 512:(do + 1) * 512],
                              osb[:])

        # ---------------- schedule ----------------
        # Loads ordered so the first scores+exp are unblocked ASAP: the
        # critical DMA chain is xtq-T, wq, wk, xtk-half0-T.
        qproj_load(0)
        wq_sb = load_weight(wq, DIN, DC, "wq_sb")
        bq_sb = const.tile([P, NDO], F32)
        nc.sync.dma_start(bq_sb[:], bq[:].rearrange("(o p) -> p o", p=P))
        qproj_chain(0, 0)
        wk_sb = load_weight(wk, DIN, DC, "wk_sb")
        bk_sb = const.tile([P, NDO], F32)
        nc.sync.dma_start(bk_sb[:], bk[:].rearrange("(o p) -> p o", p=P))

        # Phase B: K-proj, with sqt0 hp0 (and first hp1) scores+exp fused in.
        for st_i in range(4):
            xt = kproj_do0(st_i, split=(st_i == 0))
            if st_i == 0:
                scores_exp(0, 0, 2)
                scores_exp(0, 0, 3)
                kproj_rest(xt, 0, dos=(1,))
                for do in range(1, NDO):
                    qproj_chain(0, do)
                kproj_rest(xt, 0, dos=(2, 3))
            else:
                for kt in range(4 * st_i, 4 * st_i + 4):
                    scores_exp(0, 0, kt)
                kproj_rest(xt, st_i)
            if st_i >= 2:
                for kt in range(4 * (st_i - 2), 4 * (st_i - 2) + 4):
                    scores_exp(0, 1, kt)
        # Phase C: V-proj groups; consume hp0 via AV-T as each V chunk
        # lands, keep the exp stream fed with hp1/hp2 scores.
        wv_sb = load_weight(wv, DIN, DC, "wv_sb")
        bv_st = const.tile([1, DC], F32)
        nc.sync.dma_start(bv_st[0:1, :], bv[:][None, :])
        bv_bc = const.tile([P, DC], F32)
        nc.gpsimd.partition_broadcast(bv_bc[:], bv_st[0:1, :])
        wo_sb = load_weight(wo, DC, DOUT, "wo_sb")
        C_SCORES = [(1, kt) for kt in range(8, 16)] + \
                   [(2, kt) for kt in range(0, 14)]
        for st_i in range(4):
            def consume(chunk):
                av_t(0, 0, chunk)
                npop = 2 if chunk < 8 else 1
                for _ in range(npop):
                    if C_SCORES:
                        hp_n, kt = C_SCORES.pop(0)
                        scores_exp(0, hp_n, kt)
            vproj_chunk(st_i, after_sc=consume)
        # Phase D: finish sqt0 (hp1..hp3), qproj(1) as filler.
        qproj_load(1)
        filler = deque()
        for do in range(NDO):
            filler.append((qproj_chain, (1, do)))
        norm_transp(0, 0)
        D_SCORES = [(2, 14), (2, 15)] + [(3, kt) for kt in range(NKT)]
        for kt in range(NKT):
            av_t(0, 1, kt)
            for _ in range(2 if 2 <= kt < 4 else 1):
                if D_SCORES:
                    hp_n, kt_n = D_SCORES.pop(0)
                    scores_exp(0, hp_n, kt_n, dve=(kt_n % 2 == 1))
            if kt % 4 == 1 and filler:
                f, a = filler.popleft()
                f(*a)
        norm_transp(0, 1)
        for kt in range(NKT):
            av_t(0, 2, kt)
            if kt % 4 == 1 and filler:
                f, a = filler.popleft()
                f(*a)
        norm_transp(0, 2)
        for kt in range(NKT - 1):
            av_t(0, 3, kt)
            if kt % 4 == 1 and filler:
                f, a = filler.popleft()
                f(*a)
        while filler:
            f, a = filler.popleft()
            f(*a)

        # Phase E: steady flattened stream over (hp, kt); each hp's last
        # AV-T and normalize fire after the NEXT hp's first scores so the
        # exp stream never waits on the norm/transpose block. Previous
        # sqt's out-proj and next sqt's Q-proj interleave as PE filler.
        # AV-T at lag-2: scores(kt) waiting on the st-buffer rotation
        # already implies exp(kt-2) completed, so an AV-T emitted two score
        # steps behind never head-of-line blocks the PE queue.
        pend = deque([(0, 3, NKT - 1)])  # phase-D leftover AV-T
        for sqt in range(1, NSQT):
            last = sqt == NSQT - 1
            if not last:
                qproj_load(sqt + 1)
            filler = deque()
            if not last:
                for do in range(NDO):
                    for half in range(2):
                        filler.append((qproj_half, (sqt + 1, do, half)))
            for do in range(DOUT // 512):
                for sc in range(NSQC):
                    filler.append((oproj_chain, (sqt - 1, do, sc)))
            for hp in range(H // 2):
                for kt in range(NKT):
                    scores_exp(sqt, hp, kt, dve=(kt % 2 == 1))
                    pend.append((sqt, hp, kt))
                    if len(pend) > 2:
                        done = pend.popleft()
                        av_t(*done)
                        if done[2] == NKT - 1:
                            norm_transp(done[0], done[1])
                    if kt % 4 == 1 and filler:
                        f, a = filler.popleft()
                        f(*a)
            while filler:
                f, a = filler.popleft()
                f(*a)
        while pend:
            done = pend.popleft()
            av_t(*done)
            if done[2] == NKT - 1 and done != (NSQT - 1, H // 2 - 1, NKT - 1):
                norm_transp(done[0], done[1])
        norm_transp(NSQT - 1, H // 2 - 1, oproj_after_qc=True)

    nc.compile()
    return nc


# ---------------------------------------------------------------------------
# Host-side wrapper: shard across 8 NeuronCores, run SPMD, gather.
# Core c handles batch b = c // 2 and head-group g = c % 2 (8 of 16 heads,
# i.e. columns [g*512, (g+1)*512) of Wq/Wk/Wv and rows of Wo).
# ---------------------------------------------------------------------------

import numpy as np
import ml_dtypes

from concourse.bass_utils import run_bass_kernel_spmd

_NC = None
_BF16 = ml_dtypes.bfloat16


def _get_nc():
    global _NC
    if _NC is None:
        _NC = build_mha_core(S=2048, DIN=1024, DC=512, DOUT=1024, H=8,
                             depth=64, num_devices=8)
    return _NC


def _in_maps(q, k, v, Wq, bq, Wk, bk, Wv, bv, Wo, bo):
    f32 = np.float32
    maps = []
    qb = [np.ascontiguousarray(np.asarray(q[b], dtype=f32).astype(_BF16))
          for b in range(4)]
    kb = [np.ascontiguousarray(np.asarray(k[b], dtype=f32).astype(_BF16))
          for b in range(4)]
    vb = [np.ascontiguousarray(np.asarray(v[b], dtype=f32).astype(_BF16))
          for b in range(4)]
    Wq = np.asarray(Wq, dtype=f32)
    Wk = np.asarray(Wk, dtype=f32)
    Wv = np.asarray(Wv, dtype=f32)
    Wo = np.asarray(Wo, dtype=f32)
    for c in range(8):
        b, g = c // 2, c % 2
        sl = slice(g * 512, (g + 1) * 512)
        maps.append({
            "xq": qb[b],
            "xk": kb[b],
            "xv": vb[b],
            "wq": np.ascontiguousarray(Wq[:, sl].astype(_BF16)),
            "wk": np.ascontiguousarray(Wk[:, sl].astype(_BF16)),
            "wv": np.ascontiguousarray(Wv[:, sl].astype(_BF16)),
            "wo": np.ascontiguousarray(Wo[sl, :].astype(_BF16)),
            "bq": np.ascontiguousarray(bq[sl], dtype=f32),
            "bk": np.ascontiguousarray(bk[sl], dtype=f32),
            "bv": np.ascontiguousarray(bv[sl], dtype=f32),
        })
    return maps


def _gather(results, bo):
    out = np.empty((4, 2048, 1024), dtype=np.float32)
    bo32 = np.asarray(bo, dtype=np.float32)
    for b in range(4):
        out[b] = results[2 * b]["out"] + results[2 * b + 1]["out"] + bo32
    return out


def kernel(q, k, v, Wq, bq, Wk, bk, Wv, bv, Wo, bo, _trace=False):
    nc = _get_nc()
    res = run_bass_kernel_spmd(
        nc, _in_maps(q, k, v, Wq, bq, Wk, bk, Wv, bv, Wo, bo),
        core_ids=list(range(8)), trace=_trace)
    out = _gather(res.results, bo)
    if _trace:
        kernel.last_results = res
    return out


# revision 44
# speedup vs baseline: 1.0381x; 1.0137x over previous
"""Bass/Tile multi-head attention kernel builder for TRN2 (v3).

Per-core problem (core c handles batch b=c//2, head-group g=c%2):
  inputs:  xq, xk, xv [S, DIN] bf16     (batch b slices of q/k/v, host-cast)
           wq, wk, wv [DIN, DC] bf16    (column slice for this head group)
           wo [DC, DOUT] bf16           (row slice)
           bq, bk, bv [DC] f32
  output:  out [S, DOUT] f32  partial:  host sums the two head-group partials
           per batch and adds bo.

Math (per head h of H local heads, depth=64):
  xt   = X^T via DMA-xbar transpose loads          [DIN(p-major blocks), S]
  QT   = (wq_blk.T @ xt) + bq                       [DC, S]  f32r
  KT   = (wk_blk.T @ xt) + bk                       [DC, S]  f32r
  V    = (xt_chunk.T @ wv) + bv (+ ones col)        [S, DC(+1/head)] bf16
  ST   = KT_h.T @ QT_h   (64-partition contraction) [keys, q] per head
  E    = exp(ST * 1/sqrt(depth))  -> bf16           (logits O(10), no max-sub)
  OT   = E_chunk.T @ V_aug_h  (transposed-AV)       [q, depth+1] accum over keys
  O    = OT[:, :depth] / OT[:, depth]  -> bf16      (free-dim normalize)
  OTn  = O^T per head pair (PE transpose)           [DC, S] bf16
  out  = OTn.T @ wo                                 [S, DOUT] f32

The emission order is a hand-rolled software pipeline: the scalar engine
(exp over all S^2 logits) is the throughput floor, so score/exp work is
interleaved into the K/V projection phases and the per-sqt out/Q
projections are spread as PE filler inside the attention kt loop, keeping
both PE and ACT continuously fed.
"""

import math
from collections import deque
from contextlib import ExitStack

import concourse.mybir as mybir
from concourse import bacc
from concourse.masks import make_identity
from concourse.tile import TileContext

F32 = mybir.dt.float32
F32R = mybir.dt.float32r
BF16 = mybir.dt.bfloat16
P = 128
EXP = mybir.ActivationFunctionType.Exp


def build_mha_core(S=2048, DIN=1024, DC=512, DOUT=1024, H=8, depth=64,
                   SQT=512, num_devices=1, ablate="", q_bufs=2, ex_bufs=28,
                   st_bufs=4, xt_bufs=2):
    ablate = set(ablate.split(",")) if ablate else set()
    assert DC == H * depth and DC % P == 0 and DIN % P == 0 and S % SQT == 0
    NKT = S // P          # key chunks of 128
    NDIN = DIN // P       # input-dim k-tiles
    NDO = DC // P         # d_core blocks
    NSQT = S // SQT       # attention q tiles
    NSQC = SQT // P       # 128-query chunks per sqt
    scale = 1.0 / float(depth) ** 0.5

    nc = bacc.Bacc("TRN2", target_bir_lowering=False, debug=False,
                   num_devices=num_devices)
    xq = nc.dram_tensor("xq", [S, DIN], BF16, kind="ExternalInput")
    xk = nc.dram_tensor("xk", [S, DIN], BF16, kind="ExternalInput")
    xv = nc.dram_tensor("xv", [S, DIN], BF16, kind="ExternalInput")
    wq = nc.dram_tensor("wq", [DIN, DC], BF16, kind="ExternalInput")
    wk = nc.dram_tensor("wk", [DIN, DC], BF16, kind="ExternalInput")
    wv = nc.dram_tensor("wv", [DIN, DC], BF16, kind="ExternalInput")
    wo = nc.dram_tensor("wo", [DC, DOUT], BF16, kind="ExternalInput")
    bq = nc.dram_tensor("bq", [DC], F32, kind="ExternalInput")
    bk = nc.dram_tensor("bk", [DC], F32, kind="ExternalInput")
    bv = nc.dram_tensor("bv", [DC], F32, kind="ExternalInput")
    out = nc.dram_tensor("out", [S, DOUT], F32, kind="ExternalOutput")

    with TileContext(nc) as tc, ExitStack() as ctx:
        const = ctx.enter_context(tc.tile_pool(name="const", bufs=1))
        wpool = ctx.enter_context(tc.tile_pool(name="wpool", bufs=1))
        kvpool = ctx.enter_context(tc.tile_pool(name="kv", bufs=1))
        xtkv = ctx.enter_context(tc.tile_pool(name="xtkv", bufs=xt_bufs))
        xtq = ctx.enter_context(tc.tile_pool(name="xtq", bufs=xt_bufs))
        qpool = ctx.enter_context(tc.tile_pool(name="qp", bufs=q_bufs))
        expool = ctx.enter_context(tc.tile_pool(name="ex", bufs=ex_bufs))
        opool = ctx.enter_context(tc.tile_pool(name="op", bufs=2))
        misc = ctx.enter_context(tc.tile_pool(name="misc", bufs=2))
        osbp = ctx.enter_context(tc.tile_pool(name="osbp", bufs=4))
        ps_st = ctx.enter_context(tc.tile_pool(name="ps_st", bufs=st_bufs,
                                               space="PSUM"))
        ps_ot = ctx.enter_context(tc.tile_pool(name="ps_ot", bufs=1,
                                               space="PSUM"))
        ps_gen = ctx.enter_context(tc.tile_pool(name="ps_gen", bufs=2,
                                                space="PSUM"))

        ident = const.tile([P, P], BF16)
        make_identity(nc, ident)
        # warm the Exp activation table while the first DMAs are in flight
        warm = const.tile([1, 2], F32)
        nc.vector.memset(warm[:], 0.0)
        nc.scalar.activation(warm[0:1, 0:1], warm[0:1, 1:2], EXP)

        # ---- weights: direct bf16 DMA loads, no staging ----
        def load_weight(dram, kdim, ndim, name, split=False):
            w = wpool.tile([P, kdim // P, ndim], BF16, name=name)
            if split:
                # first output block loads first: unblocks the do=0 chain
                nc.sync.dma_start(
                    w[:, :, 0:P],
                    dram[:, 0:P].rearrange("(o p) n -> p o n", p=P))
                nc.sync.dma_start(
                    w[:, :, P:ndim],
                    dram[:, P:].rearrange("(o p) n -> p o n", p=P))
            else:
                nc.sync.dma_start(
                    w[:], dram[:, :].rearrange("(o p) n -> p o n", p=P))
            return w

        KT = kvpool.tile([P, NDO, S], F32R)
        V = kvpool.tile([P, NKT, H, depth + 1], BF16)
        nc.vector.memset(V[:, :, :, depth:depth + 1], 1.0)

        # ---------------- emitters ----------------
        def kproj_half(xt, st_i, do, half):
            ps = ps_gen.tile([P, 256], F32, tag="gen", name="pskh")
            for kt in range(NDIN):
                nc.tensor.matmul(
                    ps[:], wk_sb[:, kt, do * P:(do + 1) * P],
                    xt[:, kt, half * 256:(half + 1) * 256],
                    start=(kt == 0), stop=(kt == NDIN - 1))
            nc.vector.tensor_scalar_add(
                KT[:, do, st_i * 512 + half * 256:st_i * 512 + half * 256
                   + 256], ps[:], bk_sb[:, do:do + 1])

        def kproj_do0(st_i, split=False):
            xt = xtkv.tile([P, NDIN, 512], BF16, tag="xt", name="xtk")
            rows = xk[st_i * 512:(st_i + 1) * 512, :]
            if split:
                # row-split so the first scores unblock after half a chunk
                nc.sync.dma_start_transpose(xt[:, :, 0:256], rows[0:256, :])
                kproj_half(xt, st_i, 0, 0)
                scores_exp(0, 0, 4 * st_i)
                scores_exp(0, 0, 4 * st_i + 1)
                nc.sync.dma_start_transpose(xt[:, :, 256:512],
                                            rows[256:512, :])
                kproj_half(xt, st_i, 0, 1)
            else:
                nc.sync.dma_start_transpose(xt[:], rows)
                kproj_rest(xt, st_i, dos=(0,))
            return xt

        def kproj_rest(xt, st_i, dos=(1, 2, 3)):
            for do in dos:
                ps = ps_gen.tile([P, 512], F32, tag="gen", name="psk")
                for kt in range(NDIN):
                    nc.tensor.matmul(
                        ps[:], wk_sb[:, kt, do * P:(do + 1) * P], xt[:, kt, :],
                        start=(kt == 0), stop=(kt == NDIN - 1))
                nc.vector.tensor_scalar_add(
                    KT[:, do, st_i * 512:(st_i + 1) * 512], ps[:],
                    bk_sb[:, do:do + 1])

        def vproj_chunk(st_i, after_sc=None):
            xt = xtkv.tile([P, NDIN, 512], BF16, tag="xt", name="xtv")
            nc.sync.dma_start_transpose(
                xt[:], xv[st_i * 512:(st_i + 1) * 512, :])
            for sc in range(4):
                ps = ps_gen.tile([P, 512], F32, tag="gen", name="psv")
                for kt in range(NDIN):
                    nc.tensor.matmul(
                        ps[:], xt[:, kt, sc * P:(sc + 1) * P], wv_sb[:, kt, :],
                        start=(kt == 0), stop=(kt == NDIN - 1))
                chunk = st_i * 4 + sc
                nc.vector.tensor_tensor(
                    V[:, chunk, :, 0:depth],
                    ps[:].rearrange("p (h d) -> p h d", h=H),
                    bv_bc[:].rearrange("p (h d) -> p h d", h=H),
                    mybir.AluOpType.add)
                if after_sc is not None:
                    after_sc(chunk)

        QTs = {}

        def qproj_load(sqt, split_first=False):
            xt = xtq.tile([P, NDIN, SQT], BF16, tag="xt", name="xtq")
            rows = xq[sqt * SQT:(sqt + 1) * SQT, :]
            if split_first:
                # load the first k-tile separately so chain kt=0 can start
                # before the bulk of the transpose-load finishes
                nc.sync.dma_start_transpose(xt[:, 0:1, :], rows[:, 0:P])
                nc.sync.dma_start_transpose(xt[:, 1:NDIN, :], rows[:, P:])
            else:
                nc.sync.dma_start_transpose(xt[:], rows)
            QTs[sqt] = (qpool.tile([P, NDO, SQT], F32R, tag="qt", name="qt"),
                        xt)

        def qproj_chain(sqt, do):
            QT, xt = QTs[sqt]
            ps = ps_gen.tile([P, 512], F32, tag="gen", name="psq")
            for kt in range(NDIN):
                nc.tensor.matmul(
                    ps[:], wq_sb[:, kt, do * P:(do + 1) * P], xt[:, kt, :],
                    start=(kt == 0), stop=(kt == NDIN - 1))
            nc.vector.tensor_scalar_add(QT[:, do, :], ps[:],
                                        bq_sb[:, do:do + 1])

        def qproj_half(sqt, do, half):
            # finer-grained filler: half the free dim per chain
            QT, xt = QTs[sqt]
            ps = ps_gen.tile([P, 256], F32, tag="gen", name="psqh")
            for kt in range(NDIN):
                nc.tensor.matmul(
                    ps[:], wq_sb[:, kt, do * P:(do + 1) * P],
                    xt[:, kt, half * 256:(half + 1) * 256],
                    start=(kt == 0), stop=(kt == NDIN - 1))
            nc.vector.tensor_scalar_add(
                QT[:, do, half * 256:(half + 1) * 256], ps[:],
                bq_sb[:, do:do + 1])

        ex_map = {}
        ot_map = {}
        OTns = {}

        # Schraudolph bit-trick exp for the DVE: exp(s*x) ~=
        # bitcast_bf16(int16(A*x + B)) with A = 128*s/ln2, B = 127*128 - c.
        # ~+-3% per weight, self-consistent through the softmax denominator
        # (it sums the same approximated values). Used on a fraction of key
        # tiles to offload the scalar engine, which is the throughput floor.
        EXPA = 128.0 * scale / math.log(2.0)
        EXPB = 127.0 * 128.0 - 7.42

        def scores_exp(sqt, hp, kt, dve=False):
            # per-head one-bank st tiles: a 4-slot rotation in the same 4
            # PSUM banks doubles the scores->exp pipeline elasticity, and
            # per-head exp ops allow a finer ACT/DVE split.
            QT = QTs[sqt][0]
            ex = expool.tile([P, 2, 512], BF16, tag="ex", name="ex")
            for hi, h in enumerate((2 * hp, 2 * hp + 1)):
                st = ps_st.tile([P, 512], F32, name="st")
                p0 = (h % 2) * 64
                nc.tensor.matmul(
                    st[:],
                    KT[p0:p0 + 64, hp, kt * P:(kt + 1) * P],
                    QT[p0:p0 + 64, hp, :],
                    start=True, stop=True)
                on_dve = dve and (hi == 1 or kt % 4 == 3)
                if on_dve:
                    nc.vector.tensor_scalar(
                        ex[:, hi, :].bitcast(mybir.dt.int16), st[:],
                        EXPA, EXPB,
                        mybir.AluOpType.mult, mybir.AluOpType.add)
                else:
                    nc.scalar.activation(ex[:, hi, :], st[:], EXP,
                                         scale=scale)
            ex_map[(sqt, hp, kt)] = ex

        def av_t(sqt, hp, kt):
            if kt == 0:
                ot_map[(sqt, hp)] = [
                    ps_ot.tile([P, NSQC, P], F32, name=f"ot{i}")
                    for i in range(2)]
            ot_ps = ot_map[(sqt, hp)]
            ex = ex_map.pop((sqt, hp, kt))
            for hi, h in enumerate((2 * hp, 2 * hp + 1)):
                for qc in range(NSQC):
                    # start zeroes the whole 2KB PSUM zero-region (bank), so
                    # only the very first matmul into each head's bank starts
                    # the group; all four qc regions then accumulate onto
                    # zeros.
                    nc.tensor.matmul(
                        ot_ps[hi][:, qc, 0:depth + 1],
                        ex[:, hi, qc * P:(qc + 1) * P],
                        V[:, kt, h, :],
                        start=(kt == 0 and qc == 0),
                        stop=(kt == NKT - 1 and qc == NSQC - 1),
                        skip_group_check=True)

        def norm_transp(sqt, hp, oproj_after_qc=False):
            if hp == 0:
                OTns[sqt] = opool.tile([P, NDO, SQT], BF16, tag="otn",
                                       name="otn")
            OTn = OTns[sqt]
            ot_ps = ot_map.pop((sqt, hp))
            O_sb = opool.tile([P, NSQC, 2, depth], BF16, tag="osb",
                              name="osb")
            for hi in range(2):
                rec = misc.tile([P, NSQC, 1], F32, tag="rec", name="rec")
                nc.vector.reciprocal(rec[:],
                                     ot_ps[hi][:, :, depth:depth + 1])
                nc.vector.tensor_tensor(
                    O_sb[:, :, hi, :], ot_ps[hi][:, :, 0:depth],
                    rec[:].to_broadcast((P, NSQC, depth)),
                    mybir.AluOpType.mult)
            # all four 128-wide transposes accumulate into one PSUM bank
            # (start zeroes it, later ones add onto zeros), then a single
            # 512-wide copy moves the whole head-pair row into OTn.
            tp = ps_gen.tile([P, 4 * P], BF16, tag="gen", name="tp")
            for qc in range(NSQC):
                nc.tensor.matmul(
                    tp[:, qc * P:(qc + 1) * P], O_sb[:, qc, :, :], ident[:],
                    is_transpose=True, start=(qc == 0),
                    stop=(qc == NSQC - 1), skip_group_check=True)
            nc.vector.tensor_copy(OTn[:, hp, :], tp[:])
            if oproj_after_qc:
                for do in range(DOUT // 512):
                    for sc in range(NSQC):
                        oproj_chain(sqt, do, sc)

        def oproj_chain(sqt, do, sc, direct=False):
            OTn = OTns[sqt]
            ps = ps_gen.tile([P, 512], F32, tag="gen", name="pso")
            for hh in range(NDO):
                nc.tensor.matmul(
                    ps[:], OTn[:, hh, sc * P:(sc + 1) * P],
                    wo_sb[:, hh, do * 512:(do + 1) * 512],
                    start=(hh == 0), stop=(hh == NDO - 1))
            r0 = sqt * SQT + sc * P
            osb = osbp.tile([P, 512], F32, tag="osb2", name="osb2")
            nc.vector.tensor_copy(osb[:], ps[:])
            nc.sync.dma_start(out[r0:r0 + P, do * 512:(do + 1) * 512],
                              osb[:])

        # ---------------- schedule ----------------
        # Loads ordered so the first scores+exp are unblocked ASAP: the
        # critical DMA chain is xtq-T, wq, wk, xtk-half0-T.
        qproj_load(0)
        wq_sb = load_weight(wq, DIN, DC, "wq_sb")
        bq_sb = const.tile([P, NDO], F32)
        nc.sync.dma_start(bq_sb[:], bq[:].rearrange("(o p) -> p o", p=P))
        qproj_chain(0, 0)
        wk_sb = load_weight(wk, DIN, DC, "wk_sb")
        bk_sb = const.tile([P, NDO], F32)
        nc.sync.dma_start(bk_sb[:], bk[:].rearrange("(o p) -> p o", p=P))

        # Phase B: K-proj, with sqt0 hp0 (and first hp1) scores+exp fused in.
        for st_i in range(4):
            xt = kproj_do0(st_i, split=(st_i == 0))
            if st_i == 0:
                scores_exp(0, 0, 2)
                scores_exp(0, 0, 3)
                kproj_rest(xt, 0, dos=(1,))
                for do in range(1, NDO):
                    qproj_chain(0, do)
                kproj_rest(xt, 0, dos=(2, 3))
            else:
                for kt in range(4 * st_i, 4 * st_i + 4):
                    scores_exp(0, 0, kt)
                kproj_rest(xt, st_i)
            if st_i >= 2:
                for kt in range(4 * (st_i - 2), 4 * (st_i - 2) + 4):
                    scores_exp(0, 1, kt)
        # Phase C: V-proj groups; consume hp0 via AV-T as each V chunk
        # lands, keep the exp stream fed with hp1/hp2 scores.
        wv_sb = load_weight(wv, DIN, DC, "wv_sb")
        bv_st = const.tile([1, DC], F32)
        nc.sync.dma_start(bv_st[0:1, :], bv[:][None, :])
        bv_bc = const.tile([P, DC], F32)
        nc.gpsimd.partition_broadcast(bv_bc[:], bv_st[0:1, :])
        wo_sb = load_weight(wo, DC, DOUT, "wo_sb")
        C_SCORES = [(1, kt) for kt in range(8, 16)] + \
                   [(2, kt) for kt in range(0, 14)]
        for st_i in range(4):
            def consume(chunk):
                av_t(0, 0, chunk)
                npop = 2 if chunk < 8 else 1
                for _ in range(npop):
                    if C_SCORES:
                        hp_n, kt = C_SCORES.pop(0)
                        scores_exp(0, hp_n, kt)
            vproj_chunk(st_i, after_sc=consume)
        # Phase D: finish sqt0 (hp1..hp3), qproj(1) as filler.
        qproj_load(1)
        filler = deque()
        for do in range(NDO):
            filler.append((qproj_chain, (1, do)))
        norm_transp(0, 0)
        D_SCORES = [(2, 14), (2, 15)] + [(3, kt) for kt in range(NKT)]
        for kt in range(NKT):
            av_t(0, 1, kt)
            for _ in range(2 if 2 <= kt < 4 else 1):
                if D_SCORES:
                    hp_n, kt_n = D_SCORES.pop(0)
                    scores_exp(0, hp_n, kt_n, dve=(kt_n % 2 == 1))
            if kt % 4 == 1 and filler:
                f, a = filler.popleft()
                f(*a)
        norm_transp(0, 1)
        for kt in range(NKT):
            av_t(0, 2, kt)
            if kt % 4 == 1 and filler:
                f, a = filler.popleft()
                f(*a)
        norm_transp(0, 2)
        for kt in range(NKT - 1):
            av_t(0, 3, kt)
            if kt % 4 == 1 and filler:
                f, a = filler.popleft()
                f(*a)
        while filler:
            f, a = filler.popleft()
            f(*a)

        # Phase E: steady flattened stream over (hp, kt); each hp's last
        # AV-T and normalize fire after the NEXT hp's first scores so the
        # exp stream never waits on the norm/transpose block. Previous
        # sqt's out-proj and next sqt's Q-proj interleave as PE filler.
        # AV-T at lag-2: scores(kt) waiting on the st-buffer rotation
        # already implies exp(kt-2) completed, so an AV-T emitted two score
        # steps behind never head-of-line blocks the PE queue.
        pend = deque([(0, 3, NKT - 1)])  # phase-D leftover AV-T
        for sqt in range(1, NSQT):
            last = sqt == NSQT - 1
            if not last:
                qproj_load(sqt + 1)
            filler = deque()
            if not last:
                for do in range(NDO):
                    for half in range(2):
                        filler.append((qproj_half, (sqt + 1, do, half)))
            for do in range(DOUT // 512):
                for sc in range(NSQC):
                    filler.append((oproj_chain, (sqt - 1, do, sc)))
            for hp in range(H // 2):
                for kt in range(NKT):
                    scores_exp(sqt, hp, kt, dve=(kt % 2 == 1))
                    pend.append((sqt, hp, kt))
                    if len(pend) > 2:
                        done = pend.popleft()
                        av_t(*done)
                        if done[2] == NKT - 1:
                            norm_transp(done[0], done[1])
                    if kt % 4 == 1 and filler:
                        f, a = filler.popleft()
                        f(*a)
            while filler:
                f, a = filler.popleft()
                f(*a)
        while pend:
            done = pend.popleft()
            av_t(*done)
            if done[2] == NKT - 1 and done != (NSQT - 1, H // 2 - 1, NKT - 1):
                norm_transp(done[0], done[1])
        norm_transp(NSQT - 1, H // 2 - 1, oproj_after_qc=True)

    nc.compile()
    return nc


# ---------------------------------------------------------------------------
# Host-side wrapper: shard across 8 NeuronCores, run SPMD, gather.
# Core c handles batch b = c // 2 and head-group g = c % 2 (8 of 16 heads,
# i.e. columns [g*512, (g+1)*512) of Wq/Wk/Wv and rows of Wo).
# ---------------------------------------------------------------------------

import numpy as np
import ml_dtypes

from concourse.bass_utils import run_bass_kernel_spmd

_NC = None
_BF16 = ml_dtypes.bfloat16


def _get_nc():
    global _NC
    if _NC is None:
        _NC = build_mha_core(S=2048, DIN=1024, DC=512, DOUT=1024, H=8,
                             depth=64, num_devices=8)
    return _NC


def _in_maps(q, k, v, Wq, bq, Wk, bk, Wv, bv, Wo, bo):
    f32 = np.float32
    maps = []
    qb = [np.ascontiguousarray(np.asarray(q[b], dtype=f32).astype(_BF16))
          for b in range(4)]
    kb = [np.ascontiguousarray(np.asarray(k[b], dtype=f32).astype(_BF16))
          for b in range(4)]
    vb = [np.ascontiguousarray(np.asarray(v[b], dtype=f32).astype(_BF16))
          for b in range(4)]
    Wq = np.asarray(Wq, dtype=f32)
    Wk = np.asarray(Wk, dtype=f32)
    Wv = np.asarray(Wv, dtype=f32)
    Wo = np.asarray(Wo, dtype=f32)
    for c in range(8):
        b, g = c // 2, c % 2
        sl = slice(g * 512, (g + 1) * 512)
        maps.append({
            "xq": qb[b],
            "xk": kb[b],
            "xv": vb[b],
            "wq": np.ascontiguousarray(Wq[:, sl].astype(_BF16)),
            "wk": np.ascontiguousarray(Wk[:, sl].astype(_BF16)),
            "wv": np.ascontiguousarray(Wv[:, sl].astype(_BF16)),
            "wo": np.ascontiguousarray(Wo[sl, :].astype(_BF16)),
            "bq": np.ascontiguousarray(bq[sl], dtype=f32),
            "bk": np.ascontiguousarray(bk[sl], dtype=f32),
            "bv": np.ascontiguousarray(bv[sl], dtype=f32),
        })
    return maps


def _gather(results, bo):
    out = np.empty((4, 2048, 1024), dtype=np.float32)
    bo32 = np.asarray(bo, dtype=np.float32)
    for b in range(4):
        out[b] = results[2 * b]["out"] + results[2 * b + 1]["out"] + bo32
    return out


def kernel(q, k, v, Wq, bq, Wk, bk, Wv, bv, Wo, bo, _trace=False):
    nc = _get_nc()
    res = run_bass_kernel_spmd(
        nc, _in_maps(q, k, v, Wq, bq, Wk, bk, Wv, bv, Wo, bo),
        core_ids=list(range(8)), trace=_trace)
    out = _gather(res.results, bo)
    if _trace:
        kernel.last_results = res
    return out
